# revision 30
# baseline (speedup 1.0000x reference)
"""Trainium2 Bass kernel for AutoRegressiveAdaptiveSpectralConv2d (v2).

reference:  f = fft2(x)[..., :32, :32]
            o = einsum('btixy,tioxy->btoxy', f, R_w) * Ws_w
            o = (o * Wt/sum(Wt)).sum(t)            -> [B,1,U,32,32]
            out = ifft2(o, s=(256,256))            -> [B,1,U,256,256] complex64

Device decomposition (8 cores, single SPMD launch, fp16 data / fp32 PSUM):
  phase 1 (truncated DFT, sharded over 24 (b,t) pairs, 3/core = 96 images):
      stage A: PQT[w, (half,kx)] = x^T @ [cos|-sin]  (x chunks stationary)
      stage B: f[(i4,kx), (comp,ky)] = P@[cos|-sin] + (-Q)@[sin|cos]
      4-image groups share one PSUM bank; M=128 stage-B matmuls.
  AllToAll #1 (split in 2: btl {0,1} then btl {2}) -> ky-sharding
  phase 2 (channel mix, 128 xy/core): stationary = f vectors (M=4 b),
      streamed Wr/Wi (stored once - half the HBM of the re/im-packed form),
      4-way PSUM column tiling (tile_position) for array concurrency.
  AllToAll #2 -> (b, o-half) sharding
  phase 3 (zero-padded iFFT, 16 channels/core):
      G-stage: K=32 row-tiled x4; M packs (ch, comp, kx) so all PSUM->SBUF
      copies stay partition-aligned.  H-stage: K=64 packs (comp,kx) so each
      w-column streams once; 2-way row tiling via duplicated tables.
Dummy matmul chains keep the PE HAM clock warm across collective gaps.
Weights pre-scaled by 2^22 on host; host divides output by 2^38.
"""
import sys
import numpy as np

sys.path.insert(0, "/opt/trn_rl_repo")

import concourse.bass as bass
import concourse.bacc as bacc
import concourse.mybir as mybir
import concourse.tile as tile
from concourse import bass_utils

B, T, U, H, W = 4, 6, 32, 256, 256
MX, MY = 32, 32
NC = 8
CH_PER_CORE = 16
W_SCALE = float(2 ** 22)
OUT_DESCALE = float(2 ** 22) * float(H * W)

F16 = mybir.dt.float16
F32 = mybir.dt.float32


def _ap(t, offset, dims):
    """AP on a pool tile with explicit [step, count] dims (tile-relative)."""
    return bass.AP(t[:].tensor, offset, dims)


def build_nc():
    nc = bacc.Bacc("TRN2", target_bir_lowering=False, debug=False, num_devices=NC)

    xsh = nc.dram_tensor("xsh", [96, H * W], F16, kind="ExternalInput")
    ats_d = nc.dram_tensor("ats_d", [128, 128], F16, kind="ExternalInput")
    atq_d = nc.dram_tensor("atq_d", [128, 128], F16, kind="ExternalInput")
    wA_d = nc.dram_tensor("wA_d", [128, 8192], F16, kind="ExternalInput")
    wB_d = nc.dram_tensor("wB_d", [64, 8192], F16, kind="ExternalInput")
    cos4_d = nc.dram_tensor("cos4_d", [128, 256], F16, kind="ExternalInput")
    sin4_d = nc.dram_tensor("sin4_d", [128, 256], F16, kind="ExternalInput")
    sAB_d = nc.dram_tensor("sAB_d", [128, 1024], F16, kind="ExternalInput")
    outp = nc.dram_tensor("outp", [CH_PER_CORE, 2, H, W], F16, kind="ExternalOutput")

    with tile.TileContext(nc) as tc:
        with (
            tc.tile_pool(name="dram", bufs=1, space="DRAM") as dram,
            tc.tile_pool(name="consts", bufs=1) as consts,
            tc.tile_pool(name="xpool", bufs=3) as xpool,
            tc.tile_pool(name="pqt", bufs=3) as pqtp,
            tc.tile_pool(name="fsb", bufs=3) as fsbp,
            tc.tile_pool(name="wpool", bufs=1) as wpool,
            tc.tile_pool(name="f2", bufs=1) as f2p,
            tc.tile_pool(name="osb", bufs=1) as osbp,
            tc.tile_pool(name="p3", bufs=1) as p3p,
            tc.tile_pool(name="gsb", bufs=8) as gsbp,
            tc.tile_pool(name="outsb", bufs=3) as outsbp,
        ):
            warm_in = dram.tile([16, 64], F16)
            warm_out = dram.tile([16, 64], F16)
            fA_in = dram.tile([1024, 128], F16)
            fA_out = dram.tile([1024, 128], F16)
            fB_in = dram.tile([512, 128], F16)
            fB_out = dram.tile([512, 128], F16)
            o_in = dram.tile([32, 1024], F16)
            o_out = dram.tile([32, 1024], F16)

            # ---- consts + weights + x streaming (sync queue carries x first) ----
            dummy_sb = consts.tile([128, 512], F16)
            nc.vector.memset(dummy_sb[:], 0.25)
            ats = consts.tile([128, 128], F16)
            nc.scalar.dma_start(ats[:], ats_d[:])
            atq = consts.tile([128, 128], F16)
            nc.scalar.dma_start(atq[:], atq_d[:])
            cos4 = consts.tile([128, 256], F16)
            nc.scalar.dma_start(cos4[:], cos4_d[:])
            sin4 = consts.tile([128, 256], F16)
            nc.scalar.dma_start(sin4[:], sin4_d[:])
            sAB = consts.tile([128, 1024], F16)
            nc.scalar.dma_start(sAB[:], sAB_d[:])
            nc.scalar.dma_start(
                bass.AP(warm_in[:].tensor, 0, [[64, 16], [1, 64]]),
                _ap(dummy_sb, 0, [[512, 16], [1, 64]]),
            )

            x_tiles = []
            for k in range(6):
                x_sb = xpool.tile([128, 8192], F16, tag="x")
                nc.sync.dma_start(
                    _ap(x_sb, 0, [[8192, 128], [512, 16], [256, 2], [1, 256]]),
                    bass.AP(xsh, k * 16 * H * W,
                            [[256, 128], [H * W, 16], [128 * 256, 2], [1, 256]]),
                )
                x_tiles.append(x_sb)
            wA = wpool.tile([128, 8192], F16)
            nc.sync.dma_start(wA[:], wA_d[:])
            wB = wpool.tile([64, 8192], F16)
            nc.sync.dma_start(wB[:], wB_d[:])

            # warm up the collective control plane while phase 1 runs
            nc.gpsimd.collective_compute(
                "AllToAll", mybir.AluOpType.bypass,
                replica_groups=[list(range(NC))],
                ins=[warm_in.opt()], outs=[warm_out.opt()],
            )

            # ---- PE warm-up (HAM) ----
            pdummy_ctx = tc.tile_pool(name="pdummy", bufs=1, space="PSUM")
            pdummy = pdummy_ctx.__enter__()
            dummy_ps = pdummy.tile([128, 512], F32)
            for _ in range(30):
                nc.tensor.matmul(
                    dummy_ps[:], _ap(dummy_sb, 0, [[512, 128], [1, 128]]),
                    dummy_sb[:], start=True, stop=True)

            # ================= phase 1: truncated DFT =================
            p1a_ctx = tc.tile_pool(name="p1a", bufs=2, space="PSUM")
            p1a = p1a_ctx.__enter__()
            p1b_ctx = tc.tile_pool(name="p1b", bufs=2, space="PSUM")
            p1b = p1b_ctx.__enter__()
            eng = 0
            f_btl = []
            for j in range(3):
                fb = fsbp.tile([128, 512], F16, tag="fsb")
                f_btl.append(fb)
            for k in range(6):
                x_sb = x_tiles[k]
                j = k // 2
                for g4 in range(4):
                    g8 = (k % 2) * 4 + g4          # group index within btl
                    psum_a = p1a.tile([128, 512], F32, tag="a")
                    for i4 in range(4):
                        il = g4 * 4 + i4
                        for wc in range(2):
                            for hc in range(2):
                                # start=True clears the whole bank: only the
                                # first matmul into this tile may set it
                                nc.tensor.matmul(
                                    _ap(psum_a, i4 * 128 + wc * 64,
                                        [[512, 128], [1, 64]]),
                                    _ap(x_sb, il * 512 + hc * 256 + wc * 128,
                                        [[8192, 128], [1, 128]]),
                                    _ap(ats, hc * 64, [[128, 128], [1, 64]]),
                                    start=(i4 == 0 and wc == 0 and hc == 0),
                                    stop=(i4 == 3 and wc == 1 and hc == 1),
                                    skip_group_check=True,
                                )
                    pqt = pqtp.tile([128, 512], F16, tag="pqt")
                    for half in range(2):
                        cp = nc.scalar.copy if (eng % 2 == 0) else nc.vector.tensor_copy
                        cp(
                            _ap(pqt, half * 256,
                                [[512, 128], [128, 2], [32, 4], [1, 32]]),
                            _ap(psum_a, half * 32,
                                [[512, 128], [64, 2], [128, 4], [1, 32]]),
                        )
                        eng += 1
                    psum_b = p1b.tile([128, 64], F32, tag="b")
                    step = 0
                    for half in range(2):
                        rhs_t = ats if half == 0 else atq
                        for wc in range(2):
                            nc.tensor.matmul(
                                psum_b[:],
                                _ap(pqt, half * 256 + wc * 128,
                                    [[512, 128], [1, 128]]),
                                _ap(rhs_t, wc * 64, [[128, 128], [1, 64]]),
                                start=(step == 0), stop=(step == 3),
                            )
                            step += 1
                    cp = nc.scalar.copy if (eng % 2 == 0) else nc.vector.tensor_copy
                    cp(_ap(f_btl[j], g8 * 64, [[512, 128], [1, 64]]), psum_b[:])
                    eng += 1
                if k % 2 == 1:
                    # btl j complete: write its A2A payload (split per (g8, comp)
                    # to satisfy the 3-dim DMA AP limit)
                    for g8 in range(8):
                        for comp in range(2):
                            eng_dma = nc.scalar
                            if j < 2:
                                dst = bass.AP(
                                    fA_in[:].tensor,
                                    j * 8192 + comp * 4096 + g8 * 512,
                                    [[4, 128], [16384, 8], [1, 4]])
                            else:
                                dst = bass.AP(
                                    fB_in[:].tensor,
                                    comp * 4096 + g8 * 512,
                                    [[4, 128], [8192, 8], [1, 4]])
                            eng_dma.dma_start(
                                dst,
                                _ap(f_btl[j], g8 * 64 + comp * 32,
                                    [[512, 128], [4, 8], [1, 4]]),
                            )
                    if j == 1:
                        nc.gpsimd.collective_compute(
                            "AllToAll", mybir.AluOpType.bypass,
                            replica_groups=[list(range(NC))],
                            ins=[fA_in.opt()], outs=[fA_out.opt()],
                        )
                    elif j == 2:
                        nc.gpsimd.collective_compute(
                            "AllToAll", mybir.AluOpType.bypass,
                            replica_groups=[list(range(NC))],
                            ins=[fB_in.opt()], outs=[fB_out.opt()],
                        )
            p1b_ctx.__exit__(None, None, None)
            p1a_ctx.__exit__(None, None, None)

            # keep PE warm across A2A #1 (anchored on last f copy)
            for d in range(24):
                nc.tensor.matmul(
                    _ap(dummy_ps, 0, [[512, 64], [1, 512]]),
                    _ap(f_btl[2], 448, [[512, 128], [1, 64]]),
                    dummy_sb[:], start=True, stop=True)

            # ================= phase 2: spectral channel mixing =============
            o_sb = osbp.tile([128, 2048], F16)
            nc.vector.memset(o_sb[:], 0)
            fre_A = f2p.tile([128, 512], F16)
            fim_A = f2p.tile([128, 512], F16)
            fng_A = f2p.tile([128, 512], F16)
            fre_B = f2p.tile([64, 512], F16)
            fim_B = f2p.tile([64, 512], F16)
            fng_B = f2p.tile([64, 512], F16)
            for tb in range(4):
                p, jj = tb // 2, tb % 2
                for comp, dst in ((0, fre_A), (1, fim_A)):
                    nc.scalar.dma_start(
                        _ap(dst, tb * 32 * 512, [[512, 32], [128, 4], [1, 128]]),
                        bass.AP(fA_out[:].tensor,
                                (p * 128 + jj * 64 + comp * 32) * 128,
                                [[128, 32], [32768, 4], [1, 128]]),
                    )
            for tb in range(2):
                for comp, dst in ((0, fre_B), (1, fim_B)):
                    nc.scalar.dma_start(
                        _ap(dst, tb * 32 * 512, [[512, 32], [128, 4], [1, 128]]),
                        bass.AP(fB_out[:].tensor,
                                (tb * 64 + comp * 32) * 128,
                                [[128, 32], [16384, 4], [1, 128]]),
                    )
            nc.vector.tensor_scalar_mul(fng_A[:], fim_A[:], -1.0)
            nc.vector.tensor_scalar_mul(fng_B[:], fim_B[:], -1.0)

            p2_ctx = tc.tile_pool(name="p2ps", bufs=4, space="PSUM")
            p2ps = p2_ctx.__enter__()
            o_ps = []
            for _g in range(4):
                opst = p2ps.tile([128, 512], F32, tag="ops")
                nc.vector.memset(opst[:], 0)
                o_ps.append(opst)
            for rnd, (fre, fng, fim, w_t, kk) in enumerate((
                    (fre_A, fng_A, fim_A, wA, 128),
                    (fre_B, fng_B, fim_B, wB, 64))):
                for g in range(32):
                    for j4 in range(4):
                        xy = g * 4 + j4
                        lre = _ap(fre, xy, [[512, kk], [128, 4]])
                        lng = _ap(fng, xy, [[512, kk], [128, 4]])
                        lim = _ap(fim, xy, [[512, kk], [128, 4]])
                        rw = _ap(w_t, xy * 64, [[8192, kk], [1, 64]])
                        rwi = _ap(w_t, xy * 64 + 32, [[8192, kk], [1, 32]])
                        rwr = _ap(w_t, xy * 64, [[8192, kk], [1, 32]])
                        ot = o_ps[g // 8]
                        oc = 32 * j4 * 512 + (g % 8) * 64
                        tp = (0, 32 * j4)
                        # start=True clears the WHOLE psum bank, so only the
                        # very first matmul touching each bank may set it.
                        bank_first = (rnd == 0 and g % 8 == 0 and j4 == 0)
                        bank_last = (rnd == 1 and g % 8 == 7 and j4 == 3)
                        nc.tensor.matmul(
                            _ap(ot, oc, [[512, 4], [1, 64]]),
                            lre, rw, start=bank_first, stop=False,
                            tile_position=tp, skip_group_check=True)
                        nc.tensor.matmul(
                            _ap(ot, oc, [[512, 4], [1, 32]]),
                            lng, rwi, start=False, stop=False,
                            tile_position=tp, skip_group_check=True)
                        nc.tensor.matmul(
                            _ap(ot, oc + 32, [[512, 4], [1, 32]]),
                            lim, rwr, start=False, stop=bank_last,
                            tile_position=tp, skip_group_check=True)
            # o_sb cols: g*64 + oh*32 + comp*16 + ol  (oh-major so each dst
            # block reads one contiguous 32-col run per (j4, g)).  The psum
            # banks were memset, so full-width copies are race-clean.
            for gb in range(4):
                for oh in range(2):
                    cp = (nc.scalar.copy if (gb + oh) % 2 == 0
                          else nc.vector.tensor_copy)
                    cp(_ap(o_sb, gb * 512 + oh * 32,
                           [[2048, 128], [64, 8], [16, 2], [1, 16]]),
                       _ap(o_ps[gb], oh * 16,
                           [[512, 128], [64, 8], [32, 2], [1, 16]]))
            p2_ctx.__exit__(None, None, None)
            # o_in block for dst d: [4 rows (j4), 1024 cols (g, comp|ol)].
            # The (b, oh) selection happens here on the send side (SPMD-safe);
            # single-partition reads because partition-strided APs are illegal.
            for d in range(NC):
                b, oh = d // 2, d % 2
                for j4 in range(4):
                    eng_dma = nc.sync if (d + j4) % 2 == 0 else nc.scalar
                    eng_dma.dma_start(
                        bass.AP(o_in[:].tensor, d * 4096 + j4 * 1024,
                                [[1024, 1], [32, 32], [1, 32]]),
                        _ap(o_sb, (32 * j4 + b) * 2048 + oh * 32,
                            [[2048, 1], [64, 32], [1, 32]]),
                    )
            nc.gpsimd.collective_compute(
                "AllToAll", mybir.AluOpType.bypass,
                replica_groups=[list(range(NC))],
                ins=[o_in.opt()], outs=[o_out.opt()],
            )

            # keep PE warm across A2A #2 (anchored on last o copies)
            for d in range(40):
                nc.tensor.matmul(
                    _ap(dummy_ps, 0, [[512, 64], [1, 512]]),
                    _ap(o_sb, 1984, [[2048, 128], [1, 64]]),
                    dummy_sb[:], start=True, stop=True)

            # ================= phase 3: zero-padded iFFT ====================
            # o_out rows: s*4 + j4(=kyl) ; cols g(=kx)*32 + comp*16 + ol
            o3a = p3p.tile([32, 1024], F16)
            for s in range(NC):
                eng_dma = nc.sync if s % 2 == 0 else nc.scalar
                eng_dma.dma_start(
                    _ap(o3a, 4 * s * 1024, [[1024, 4], [32, 32], [16, 2], [1, 16]]),
                    bass.AP(o_out[:].tensor, s * 4096,
                            [[1024, 4], [32, 32], [16, 2], [1, 16]]),
                )
            o3r = p3p.tile([128, 1024], F16)
            for jg in range(4):
                eng_dma = nc.sync if jg % 2 == 0 else nc.scalar
                eng_dma.dma_start(
                    _ap(o3r, jg * 32 * 1024, [[1024, 32], [1, 1024]]),
                    _ap(o3a, 0, [[1024, 32], [1, 1024]]),
                )
            st_p = p3p.tile([128, 1024], F16)
            st_q = p3p.tile([128, 1024], F16)
            for cp_i in range(8):
                jg = cp_i % 4
                base = jg * 32 * 1024
                # st_p: (ch, compG, kx) <- comp_src = compG
                cpe = nc.scalar.copy if (cp_i % 2 == 0) else nc.vector.tensor_copy
                cpe(
                    _ap(st_p, base + cp_i * 128,
                        [[1024, 32], [64, 2], [32, 2], [1, 32]]),
                    _ap(o3r, base + 2 * cp_i,
                        [[1024, 32], [1, 2], [16, 2], [32, 32]]),
                )
                # st_q compG=0 half: -Oi
                nc.scalar.mul(
                    _ap(st_q, base + cp_i * 128,
                        [[1024, 32], [64, 2], [1, 32]]),
                    _ap(o3r, base + 2 * cp_i + 16,
                        [[1024, 32], [1, 2], [32, 32]]),
                    -1.0,
                )
                # st_q compG=1 half: +Or
                nc.vector.tensor_copy(
                    _ap(st_q, base + cp_i * 128 + 32,
                        [[1024, 32], [64, 2], [1, 32]]),
                    _ap(o3r, base + 2 * cp_i,
                        [[1024, 32], [1, 2], [32, 32]]),
                )
            p3g_ctx = tc.tile_pool(name="p3g", bufs=4, space="PSUM")
            p3g = p3g_ctx.__enter__()
            p3h_ctx = tc.tile_pool(name="p3h", bufs=3, space="PSUM")
            p3h = p3h_ctx.__enter__()
            g_tiles = []
            for cp_i in range(8):
                jg = cp_i % 4
                base = jg * 32 * 1024
                gp = p3g.tile([128, 256], F32, tag="gp")
                nc.tensor.matmul(
                    gp[:],
                    _ap(st_p, base + cp_i * 128, [[1024, 32], [1, 128]]),
                    _ap(cos4, jg * 32 * 256, [[256, 32], [1, 256]]),
                    start=True, stop=False, tile_position=(32 * jg, 0))
                nc.tensor.matmul(
                    gp[:],
                    _ap(st_q, base + cp_i * 128, [[1024, 32], [1, 128]]),
                    _ap(sin4, jg * 32 * 256, [[256, 32], [1, 256]]),
                    start=False, stop=True, tile_position=(32 * jg, 0))
                g_sb = gsbp.tile([128, 256], F16, tag="gsb")
                cpe = nc.scalar.copy if (cp_i % 2 == 0) else nc.vector.tensor_copy
                cpe(g_sb[:], gp[:])
                g_tiles.append(g_sb)
            eng = 0
            for cp_i in range(8):
                for par in range(2):
                    ch = cp_i * 2 + par
                    out_sb = outsbp.tile([128, 1024], F16, tag="osb3")
                    for hc in range(2):
                        hp = p3h.tile([128, 512], F32, tag="hp")
                        for variant in range(2):
                            nc.tensor.matmul(
                                _ap(hp, variant * 256, [[512, 128], [1, 256]]),
                                _ap(sAB, par * 64 * 1024 + variant * 512 + hc * 128,
                                    [[1024, 64], [1, 128]]),
                                _ap(g_tiles[cp_i], par * 64 * 256,
                                    [[256, 64], [1, 256]]),
                                start=(variant == 0), stop=(variant == 1),
                                tile_position=(par * 64, 0),
                                skip_group_check=True)
                        # out_sb cols: comp*512 + hc*256 + w (comp-major so the
                        # outp DMA merges to <=3 dims on both sides)
                        cpe = nc.scalar.copy if (eng % 2 == 0) else nc.vector.tensor_copy
                        cpe(_ap(out_sb, hc * 256, [[1024, 128], [512, 2], [1, 256]]),
                            hp[:])
                        eng += 1
                    eng_dma = nc.sync if ch % 2 == 0 else nc.scalar
                    eng_dma.dma_start(
                        bass.AP(outp, ch * 2 * H * W,
                                [[256, 128], [128 * 256, 4], [1, 256]]),
                        _ap(out_sb, 0, [[1024, 128], [256, 4], [1, 256]]),
                    )
            p3h_ctx.__exit__(None, None, None)
            p3g_ctx.__exit__(None, None, None)
            pdummy_ctx.__exit__(None, None, None)
    nc.compile()
    return nc


_NC_CACHE = None


def _get_nc():
    global _NC_CACHE
    if _NC_CACHE is None:
        _NC_CACHE = build_nc()
    return _NC_CACHE


def _host_prep(x, R_w, Ws_w, Wt_w):
    x = np.asarray(x)
    R_w = np.asarray(R_w)
    Ws_w = np.asarray(Ws_w, dtype=np.float32)
    Wt_w = np.asarray(Wt_w, dtype=np.float32)
    f16, f32 = np.float16, np.float32

    xf = x.reshape(B * T, U, H, W).astype(f16)

    h = np.arange(H)[:, None]
    k = np.arange(MX)[None, :]
    ang = 2.0 * np.pi * h * k / H
    ATs = np.concatenate([np.cos(ang), -np.sin(ang)], axis=1).astype(f16)
    ATq = np.concatenate([np.sin(ang), np.cos(ang)], axis=1).astype(f16)
    ats = np.concatenate([ATs[0:128], ATs[128:256]], axis=1)   # [128, 128]
    atq = np.concatenate([ATq[0:128], ATq[128:256]], axis=1)

    wt = (Wt_w / Wt_w.sum()).reshape(T)
    Wc = (R_w * Ws_w[None, None, None]
          * wt[:, None, None, None, None].astype(f32) * W_SCALE)
    Wr = np.real(Wc).astype(f32)   # [T,U,U,MX,MY] = (t,i,o,kx,ky)
    Wi = np.imag(Wc).astype(f32)

    # w tiles: rows (tb, i); cols (kx*4+kyl)*64 + compW*32 + o ; ky = 4c + kyl
    # [T,U,U,MX,MY] -> per (t,i): [o, kx, ky]
    Wr_t = Wr.transpose(0, 1, 3, 4, 2)   # [t, i, kx, ky, o]
    Wi_t = Wi.transpose(0, 1, 3, 4, 2)
    w_A = np.empty((NC, 128, 8192), f16)
    w_B = np.empty((NC, 64, 8192), f16)
    for c in range(NC):
        kys = slice(4 * c, 4 * c + 4)
        for tb in range(4):
            p, jj = tb // 2, tb % 2
            t = 3 * p + jj
            blk = np.stack([Wr_t[t, :, :, kys, :], Wi_t[t, :, :, kys, :]], axis=3)
            # blk: [i, kx, kyl, compW, o] -> cols (kx, kyl, compW, o)
            w_A[c, tb * 32:(tb + 1) * 32] = blk.reshape(U, 8192).astype(f16)
        for tb in range(2):
            t = 3 * tb + 2
            blk = np.stack([Wr_t[t, :, :, kys, :], Wi_t[t, :, :, kys, :]], axis=3)
            w_B[c, tb * 32:(tb + 1) * 32] = blk.reshape(U, 8192).astype(f16)

    xg = np.arange(MX)[:, None]
    wg = np.arange(W)[None, :]
    ang2 = 2.0 * np.pi * xg * wg / W
    cos2 = np.cos(ang2).astype(f32)
    sin2 = np.sin(ang2).astype(f32)
    cos4 = np.tile(cos2.astype(f16), (4, 1))   # [128, 256]
    sin4 = np.tile(sin2.astype(f16), (4, 1))
    sAB = np.zeros((128, 1024), f16)
    for rep in range(2):
        r0 = rep * 64
        for hc in range(2):
            blkc = cos2[:, hc * 128:(hc + 1) * 128].astype(f16)
            blks = sin2[:, hc * 128:(hc + 1) * 128].astype(f16)
            sAB[r0:r0 + 32, hc * 128:hc * 128 + 128] = blkc
            sAB[r0 + 32:r0 + 64, hc * 128:hc * 128 + 128] = -blks
            sAB[r0:r0 + 32, 512 + hc * 128:512 + hc * 128 + 128] = blks
            sAB[r0 + 32:r0 + 64, 512 + hc * 128:512 + hc * 128 + 128] = blkc
    in_maps = []
    for c in range(NC):
        in_maps.append({
            "xsh": np.ascontiguousarray(xf[c * 3:(c + 1) * 3].reshape(96, H * W)),
            "ats_d": ats, "atq_d": atq,
            "wA_d": np.ascontiguousarray(w_A[c]),
            "wB_d": np.ascontiguousarray(w_B[c]),
            "cos4_d": cos4, "sin4_d": sin4, "sAB_d": sAB,
        })
    return in_maps


def _host_post(results):
    out = np.empty((B, 1, U, H, W), np.complex64)
    inv = np.float32(1.0 / OUT_DESCALE)
    for c in range(NC):
        arr = np.asarray(results[c]["outp"]).astype(np.float32)  # [16,2,256,256]
        carr = (arr[:, 0] + 1j * arr[:, 1]).astype(np.complex64)
        b, oh = c // 2, c % 2
        for ol in range(CH_PER_CORE):
            out[b, 0, oh * 16 + ol] = carr[ol] * inv
    return out


def kernel(**inputs):
    nc = _get_nc()
    in_maps = _host_prep(inputs["input"], inputs["R_w"], inputs["Ws_w"], inputs["Wt_w"])
    res = bass_utils.run_bass_kernel_spmd(nc, in_maps, core_ids=list(range(NC)))
    return _host_post(res.results)


# revision 34
# speedup vs baseline: 1.9460x; 1.9460x over previous
"""Trainium2 Bass kernel for AutoRegressiveAdaptiveSpectralConv2d (v2).

reference:  f = fft2(x)[..., :32, :32]
            o = einsum('btixy,tioxy->btoxy', f, R_w) * Ws_w
            o = (o * Wt/sum(Wt)).sum(t)            -> [B,1,U,32,32]
            out = ifft2(o, s=(256,256))            -> [B,1,U,256,256] complex64

Device decomposition (8 cores, single SPMD launch, fp16 data / fp32 PSUM):
  phase 1 (truncated DFT, sharded over 24 (b,t) pairs, 3/core = 96 images):
      stage A: PQT[w, (half,kx)] = x^T @ [cos|-sin]  (x chunks stationary)
      stage B: f[(i4,kx), (comp,ky)] = P@[cos|-sin] + (-Q)@[sin|cos]
      4-image groups share one PSUM bank; M=128 stage-B matmuls.
  AllToAll #1 (split in 2: btl {0,1} then btl {2}) -> ky-sharding
  phase 2 (channel mix, 128 xy/core): stationary = f vectors (M=4 b),
      streamed Wr/Wi (stored once - half the HBM of the re/im-packed form),
      4-way PSUM column tiling (tile_position) for array concurrency.
  AllToAll #2 -> (b, o-half) sharding
  phase 3 (zero-padded iFFT, 16 channels/core):
      G-stage: K=32 row-tiled x4; M packs (ch, comp, kx) so all PSUM->SBUF
      copies stay partition-aligned.  H-stage: K=64 packs (comp,kx) so each
      w-column streams once; 2-way row tiling via duplicated tables.
Dummy matmul chains keep the PE HAM clock warm across collective gaps.
Weights pre-scaled by 2^22 on host; host divides output by 2^38.
"""
import sys
import numpy as np

sys.path.insert(0, "/opt/trn_rl_repo")

import concourse.bass as bass
import concourse.bacc as bacc
import concourse.mybir as mybir
import concourse.tile as tile
from concourse import bass_utils

B, T, U, H, W = 4, 6, 32, 256, 256
MX, MY = 32, 32
NC = 8
CH_PER_CORE = 16
W_SCALE = float(2 ** 22)
OUT_DESCALE = float(2 ** 22) * float(H * W)

F16 = mybir.dt.float16
F32 = mybir.dt.float32


def _ap(t, offset, dims):
    """AP on a pool tile with explicit [step, count] dims (tile-relative)."""
    return bass.AP(t[:].tensor, offset, dims)


def build_nc():
    nc = bacc.Bacc("TRN2", target_bir_lowering=False, debug=False, num_devices=NC)

    xsh = nc.dram_tensor("xsh", [96, H * W], F16, kind="ExternalInput")
    ats_d = nc.dram_tensor("ats_d", [128, 128], F16, kind="ExternalInput")
    atsB_d = nc.dram_tensor("atsB_d", [128, 128], F16, kind="ExternalInput")
    atqB_d = nc.dram_tensor("atqB_d", [128, 128], F16, kind="ExternalInput")
    wA_d = nc.dram_tensor("wA_d", [128, 8192], F16, kind="ExternalInput")
    wB_d = nc.dram_tensor("wB_d", [64, 8192], F16, kind="ExternalInput")
    cos4_d = nc.dram_tensor("cos4_d", [128, 256], F16, kind="ExternalInput")
    sin4_d = nc.dram_tensor("sin4_d", [128, 256], F16, kind="ExternalInput")
    sAB_d = nc.dram_tensor("sAB_d", [128, 1024], F16, kind="ExternalInput")
    outp = nc.dram_tensor("outp", [CH_PER_CORE, 2, H, W], F16, kind="ExternalOutput")

    with tile.TileContext(nc) as tc:
        with (
            tc.tile_pool(name="dram", bufs=1, space="DRAM") as dram,
            tc.tile_pool(name="consts", bufs=1) as consts,
            tc.tile_pool(name="xpool", bufs=3) as xpool,
            tc.tile_pool(name="pqt", bufs=3) as pqtp,
            tc.tile_pool(name="fsb", bufs=3) as fsbp,
            tc.tile_pool(name="wpool", bufs=1) as wpool,
            tc.tile_pool(name="f2", bufs=1) as f2p,
            tc.tile_pool(name="osb", bufs=1) as osbp,
            tc.tile_pool(name="p3", bufs=1) as p3p,
            tc.tile_pool(name="gsb", bufs=8) as gsbp,
            tc.tile_pool(name="outsb", bufs=3) as outsbp,
        ):
            fA_in = dram.tile([512, 256], F16)
            fA_out = dram.tile([512, 256], F16)
            fB_in = dram.tile([256, 256], F16)
            fB_out = dram.tile([256, 256], F16)
            o_in = dram.tile([32, 1024], F16)
            o_out = dram.tile([32, 1024], F16)

            # ---- consts + weights + x streaming (sync queue carries x first) ----
            dummy_sb = consts.tile([128, 512], F16)
            nc.vector.memset(dummy_sb[:], 0.25)
            ats = consts.tile([128, 128], F16)
            nc.scalar.dma_start(ats[:], ats_d[:])
            atsB = consts.tile([128, 128], F16)
            nc.scalar.dma_start(atsB[:], atsB_d[:])
            atqB = consts.tile([128, 128], F16)
            nc.scalar.dma_start(atqB[:], atqB_d[:])
            cos4 = consts.tile([128, 256], F16)
            nc.scalar.dma_start(cos4[:], cos4_d[:])
            sin4 = consts.tile([128, 256], F16)
            nc.scalar.dma_start(sin4[:], sin4_d[:])
            sAB = consts.tile([128, 1024], F16)
            nc.scalar.dma_start(sAB[:], sAB_d[:])

            x_tiles = []
            for k in range(6):
                x_sb = xpool.tile([128, 8192], F16, tag="x")
                nc.sync.dma_start(
                    _ap(x_sb, 0, [[8192, 128], [512, 16], [256, 2], [1, 256]]),
                    bass.AP(xsh, k * 16 * H * W,
                            [[256, 128], [H * W, 16], [128 * 256, 2], [1, 256]]),
                )
                x_tiles.append(x_sb)
            wA = wpool.tile([128, 8192], F16)
            nc.sync.dma_start(wA[:], wA_d[:])
            wB = wpool.tile([64, 8192], F16)
            nc.sync.dma_start(wB[:], wB_d[:])


            # ---- PE warm-up (HAM) ----
            pdummy_ctx = tc.tile_pool(name="pdummy", bufs=1, space="PSUM")
            pdummy = pdummy_ctx.__enter__()
            dummy_ps = pdummy.tile([128, 512], F32)
            for _ in range(30):
                nc.tensor.matmul(
                    dummy_ps[:], _ap(dummy_sb, 0, [[512, 128], [1, 128]]),
                    dummy_sb[:], start=True, stop=True)

            # ================= phase 1: truncated DFT =================
            p1a_ctx = tc.tile_pool(name="p1a", bufs=2, space="PSUM")
            p1a = p1a_ctx.__enter__()
            p1b_ctx = tc.tile_pool(name="p1b", bufs=2, space="PSUM")
            p1b = p1b_ctx.__enter__()
            eng = 0
            f_btl = []
            for j in range(3):
                fb = fsbp.tile([128, 512], F16, tag="fsb")
                f_btl.append(fb)
            for k in range(6):
                x_sb = x_tiles[k]
                j = k // 2
                for g4 in range(4):
                    g8 = (k % 2) * 4 + g4          # group index within btl
                    psum_a = p1a.tile([128, 512], F32, tag="a")
                    for i4 in range(4):
                        il = g4 * 4 + i4
                        for wc in range(2):
                            for hc in range(2):
                                # start=True clears the whole bank: only the
                                # first matmul into this tile may set it
                                nc.tensor.matmul(
                                    _ap(psum_a, i4 * 128 + wc * 64,
                                        [[512, 128], [1, 64]]),
                                    _ap(x_sb, il * 512 + hc * 256 + wc * 128,
                                        [[8192, 128], [1, 128]]),
                                    _ap(ats, hc * 64, [[128, 128], [1, 64]]),
                                    start=(i4 == 0 and wc == 0 and hc == 0),
                                    stop=(i4 == 3 and wc == 1 and hc == 1),
                                    skip_group_check=True,
                                )
                    pqt = pqtp.tile([128, 512], F16, tag="pqt")
                    for half in range(2):
                        cp = nc.scalar.copy if (eng % 2 == 0) else nc.vector.tensor_copy
                        cp(
                            _ap(pqt, half * 256,
                                [[512, 128], [128, 2], [32, 4], [1, 32]]),
                            _ap(psum_a, half * 32,
                                [[512, 128], [64, 2], [128, 4], [1, 32]]),
                        )
                        eng += 1
                    psum_b = p1b.tile([128, 64], F32, tag="b")
                    step = 0
                    for half in range(2):
                        rhs_t = atsB if half == 0 else atqB
                        for wc in range(2):
                            nc.tensor.matmul(
                                psum_b[:],
                                _ap(pqt, half * 256 + wc * 128,
                                    [[512, 128], [1, 128]]),
                                _ap(rhs_t, wc * 64, [[128, 128], [1, 64]]),
                                start=(step == 0), stop=(step == 3),
                            )
                            step += 1
                    cp = nc.scalar.copy if (eng % 2 == 0) else nc.vector.tensor_copy
                    cp(_ap(f_btl[j], g8 * 64, [[512, 128], [1, 64]]), psum_b[:])
                    eng += 1
                if k % 2 == 1:
                    # btl j complete: write its A2A payload.  f_sb cols are
                    # (g8, ky, comp) with (kyl, comp) 8-elem runs per dst, so
                    # one 16KB DMA per destination suffices.
                    for dd in range(NC):
                        if j < 2:
                            dst = bass.AP(
                                fA_in[:].tensor, dd * 16384 + j * 8192,
                                [[8, 128], [1024, 8], [1, 8]])
                        else:
                            dst = bass.AP(
                                fB_in[:].tensor, dd * 8192,
                                [[8, 128], [1024, 8], [1, 8]])
                        nc.scalar.dma_start(
                            dst,
                            _ap(f_btl[j], dd * 8, [[512, 128], [64, 8], [1, 8]]),
                        )
                    if j == 1:
                        nc.gpsimd.collective_compute(
                            "AllToAll", mybir.AluOpType.bypass,
                            replica_groups=[list(range(NC))],
                            ins=[fA_in.opt()], outs=[fA_out.opt()],
                        )
                    elif j == 2:
                        nc.gpsimd.collective_compute(
                            "AllToAll", mybir.AluOpType.bypass,
                            replica_groups=[list(range(NC))],
                            ins=[fB_in.opt()], outs=[fB_out.opt()],
                        )
            p1b_ctx.__exit__(None, None, None)
            p1a_ctx.__exit__(None, None, None)

            # keep PE warm across A2A #1 (anchored on last f copy)
            for d in range(24):
                nc.tensor.matmul(
                    _ap(dummy_ps, 0, [[512, 64], [1, 512]]),
                    _ap(f_btl[2], 448, [[512, 128], [1, 64]]),
                    dummy_sb[:], start=True, stop=True)

            # ================= phase 2: spectral channel mixing =============
            o_sb = osbp.tile([128, 2048], F16)
            nc.vector.memset(o_sb[:], 0)
            fci_A = f2p.tile([128, 1024], F16)
            fng_A = f2p.tile([128, 1024], F16)
            fci_B = f2p.tile([64, 1024], F16)
            fng_B = f2p.tile([64, 1024], F16)
            for tb in range(4):
                p, jj = tb // 2, tb % 2
                nc.scalar.dma_start(
                    _ap(fci_A, tb * 32 * 1024, [[1024, 32], [256, 4], [1, 256]]),
                    bass.AP(fA_out[:].tensor, p * 16384 + jj * 8192,
                            [[256, 32], [32768, 4], [1, 256]]),
                )
            for tb in range(2):
                nc.scalar.dma_start(
                    _ap(fci_B, tb * 32 * 1024, [[1024, 32], [256, 4], [1, 256]]),
                    bass.AP(fB_out[:].tensor, tb * 8192,
                            [[256, 32], [16384, 4], [1, 256]]),
                )
            nc.vector.tensor_scalar_mul(fng_A[:], fci_A[:], -1.0)
            nc.vector.tensor_scalar_mul(fng_B[:], fci_B[:], -1.0)

            p2_ctx = tc.tile_pool(name="p2ps", bufs=4, space="PSUM")
            p2ps = p2_ctx.__enter__()
            o_ps = []
            for _g in range(4):
                opst = p2ps.tile([128, 512], F32, tag="ops")
                nc.vector.memset(opst[:], 0)
                o_ps.append(opst)
            for rnd, (fci, fng, w_t, kk) in enumerate((
                    (fci_A, fng_A, wA, 128),
                    (fci_B, fng_B, wB, 64))):
                for g in range(32):
                    for j4 in range(4):
                        xy = g * 4 + j4
                        cb = (xy // 4) * 8 + (xy % 4) * 2
                        lre = _ap(fci, cb, [[1024, kk], [256, 4]])
                        lng = _ap(fng, cb + 1, [[1024, kk], [256, 4]])
                        lim = _ap(fci, cb + 1, [[1024, kk], [256, 4]])
                        rw = _ap(w_t, xy * 64, [[8192, kk], [1, 64]])
                        rwi = _ap(w_t, xy * 64 + 32, [[8192, kk], [1, 32]])
                        rwr = _ap(w_t, xy * 64, [[8192, kk], [1, 32]])
                        ot = o_ps[g // 8]
                        oc = 32 * j4 * 512 + (g % 8) * 64
                        tp = (0, 32 * j4)
                        # start=True clears the WHOLE psum bank, so only the
                        # very first matmul touching each bank may set it.
                        bank_first = (rnd == 0 and g % 8 == 0 and j4 == 0)
                        bank_last = (rnd == 1 and g % 8 == 7 and j4 == 3)
                        nc.tensor.matmul(
                            _ap(ot, oc, [[512, 4], [1, 64]]),
                            lre, rw, start=bank_first, stop=False,
                            tile_position=tp, skip_group_check=True)
                        nc.tensor.matmul(
                            _ap(ot, oc, [[512, 4], [1, 32]]),
                            lng, rwi, start=False, stop=False,
                            tile_position=tp, skip_group_check=True)
                        nc.tensor.matmul(
                            _ap(ot, oc + 32, [[512, 4], [1, 32]]),
                            lim, rwr, start=False, stop=bank_last,
                            tile_position=tp, skip_group_check=True)
            # o_sb cols: g*64 + oh*32 + comp*16 + ol  (oh-major so each dst
            # block reads one contiguous 32-col run per (j4, g)).  The psum
            # banks were memset, so full-width copies are race-clean.
            for gb in range(4):
                for oh in range(2):
                    cp = (nc.scalar.copy if (gb + oh) % 2 == 0
                          else nc.vector.tensor_copy)
                    cp(_ap(o_sb, gb * 512 + oh * 32,
                           [[2048, 128], [64, 8], [16, 2], [1, 16]]),
                       _ap(o_ps[gb], oh * 16,
                           [[512, 128], [64, 8], [32, 2], [1, 16]]))
            p2_ctx.__exit__(None, None, None)
            # o_in block for dst d: [4 rows (j4), 1024 cols (g, comp|ol)].
            # The (b, oh) selection happens here on the send side (SPMD-safe);
            # single-partition reads because partition-strided APs are illegal.
            for d in range(NC):
                b, oh = d // 2, d % 2
                for j4 in range(4):
                    eng_dma = nc.sync if (d + j4) % 2 == 0 else nc.scalar
                    eng_dma.dma_start(
                        bass.AP(o_in[:].tensor, d * 4096 + j4 * 1024,
                                [[1024, 1], [32, 32], [1, 32]]),
                        _ap(o_sb, (32 * j4 + b) * 2048 + oh * 32,
                            [[2048, 1], [64, 32], [1, 32]]),
                    )
            nc.gpsimd.collective_compute(
                "AllToAll", mybir.AluOpType.bypass,
                replica_groups=[list(range(NC))],
                ins=[o_in.opt()], outs=[o_out.opt()],
            )

            # keep PE warm across A2A #2 (anchored on last o copies)
            for d in range(40):
                nc.tensor.matmul(
                    _ap(dummy_ps, 0, [[512, 64], [1, 512]]),
                    _ap(o_sb, 1984, [[2048, 128], [1, 64]]),
                    dummy_sb[:], start=True, stop=True)

            # ================= phase 3: zero-padded iFFT ====================
            # o_out rows: s*4 + j4(=kyl) ; cols g(=kx)*32 + comp*16 + ol
            o3a = p3p.tile([32, 1024], F16)
            for s in range(NC):
                eng_dma = nc.sync if s % 2 == 0 else nc.scalar
                eng_dma.dma_start(
                    _ap(o3a, 4 * s * 1024, [[1024, 4], [32, 32], [16, 2], [1, 16]]),
                    bass.AP(o_out[:].tensor, s * 4096,
                            [[1024, 4], [32, 32], [16, 2], [1, 16]]),
                )
            o3r = p3p.tile([128, 1024], F16)
            for jg in range(4):
                eng_dma = nc.sync if jg % 2 == 0 else nc.scalar
                eng_dma.dma_start(
                    _ap(o3r, jg * 32 * 1024, [[1024, 32], [1, 1024]]),
                    _ap(o3a, 0, [[1024, 32], [1, 1024]]),
                )
            st_p = p3p.tile([128, 1024], F16)
            st_q = p3p.tile([128, 1024], F16)
            for cp_i in range(8):
                jg = cp_i % 4
                base = jg * 32 * 1024
                # st_p: (ch, compG, kx) <- comp_src = compG
                cpe = nc.scalar.copy if (cp_i % 2 == 0) else nc.vector.tensor_copy
                cpe(
                    _ap(st_p, base + cp_i * 128,
                        [[1024, 32], [64, 2], [32, 2], [1, 32]]),
                    _ap(o3r, base + 2 * cp_i,
                        [[1024, 32], [1, 2], [16, 2], [32, 32]]),
                )
                # st_q compG=0 half: -Oi
                nc.scalar.mul(
                    _ap(st_q, base + cp_i * 128,
                        [[1024, 32], [64, 2], [1, 32]]),
                    _ap(o3r, base + 2 * cp_i + 16,
                        [[1024, 32], [1, 2], [32, 32]]),
                    -1.0,
                )
                # st_q compG=1 half: +Or
                nc.vector.tensor_copy(
                    _ap(st_q, base + cp_i * 128 + 32,
                        [[1024, 32], [64, 2], [1, 32]]),
                    _ap(o3r, base + 2 * cp_i,
                        [[1024, 32], [1, 2], [32, 32]]),
                )
            p3g_ctx = tc.tile_pool(name="p3g", bufs=4, space="PSUM")
            p3g = p3g_ctx.__enter__()
            p3h_ctx = tc.tile_pool(name="p3h", bufs=3, space="PSUM")
            p3h = p3h_ctx.__enter__()
            g_tiles = []
            for cp_i in range(8):
                jg = cp_i % 4
                base = jg * 32 * 1024
                gp = p3g.tile([128, 256], F32, tag="gp")
                nc.tensor.matmul(
                    gp[:],
                    _ap(st_p, base + cp_i * 128, [[1024, 32], [1, 128]]),
                    _ap(cos4, jg * 32 * 256, [[256, 32], [1, 256]]),
                    start=True, stop=False, tile_position=(32 * jg, 0))
                nc.tensor.matmul(
                    gp[:],
                    _ap(st_q, base + cp_i * 128, [[1024, 32], [1, 128]]),
                    _ap(sin4, jg * 32 * 256, [[256, 32], [1, 256]]),
                    start=False, stop=True, tile_position=(32 * jg, 0))
                g_sb = gsbp.tile([128, 256], F16, tag="gsb")
                cpe = nc.scalar.copy if (cp_i % 2 == 0) else nc.vector.tensor_copy
                cpe(g_sb[:], gp[:])
                g_tiles.append(g_sb)
            eng = 0
            for cp_i in range(8):
                for par in range(2):
                    ch = cp_i * 2 + par
                    out_sb = outsbp.tile([128, 1024], F16, tag="osb3")
                    for hc in range(2):
                        hp = p3h.tile([128, 512], F32, tag="hp")
                        for variant in range(2):
                            nc.tensor.matmul(
                                _ap(hp, variant * 256, [[512, 128], [1, 256]]),
                                _ap(sAB, par * 64 * 1024 + variant * 512 + hc * 128,
                                    [[1024, 64], [1, 128]]),
                                _ap(g_tiles[cp_i], par * 64 * 256,
                                    [[256, 64], [1, 256]]),
                                start=(variant == 0), stop=(variant == 1),
                                tile_position=(par * 64, 0),
                                skip_group_check=True)
                        # out_sb cols: comp*512 + hc*256 + w (comp-major so the
                        # outp DMA merges to <=3 dims on both sides)
                        cpe = nc.scalar.copy if (eng % 2 == 0) else nc.vector.tensor_copy
                        cpe(_ap(out_sb, hc * 256, [[1024, 128], [512, 2], [1, 256]]),
                            hp[:])
                        eng += 1
                    eng_dma = nc.sync if ch % 2 == 0 else nc.scalar
                    eng_dma.dma_start(
                        bass.AP(outp, ch * 2 * H * W,
                                [[256, 128], [128 * 256, 4], [1, 256]]),
                        _ap(out_sb, 0, [[1024, 128], [256, 4], [1, 256]]),
                    )
            p3h_ctx.__exit__(None, None, None)
            p3g_ctx.__exit__(None, None, None)
            pdummy_ctx.__exit__(None, None, None)
    nc.compile()
    return nc


_NC_CACHE = None


def _get_nc():
    global _NC_CACHE
    if _NC_CACHE is None:
        _NC_CACHE = build_nc()
    return _NC_CACHE


def _host_prep(x, R_w, Ws_w, Wt_w):
    x = np.asarray(x)
    R_w = np.asarray(R_w)
    Ws_w = np.asarray(Ws_w, dtype=np.float32)
    Wt_w = np.asarray(Wt_w, dtype=np.float32)
    f16, f32 = np.float16, np.float32

    xf = x.reshape(B * T, U, H, W).astype(f16)

    h = np.arange(H)[:, None]
    k = np.arange(MX)[None, :]
    ang = 2.0 * np.pi * h * k / H
    ATs = np.concatenate([np.cos(ang), -np.sin(ang)], axis=1).astype(f16)
    ats = np.concatenate([ATs[0:128], ATs[128:256]], axis=1)   # [128, 128]
    # stage-B tables have (ky, comp)-interleaved columns so that the f
    # payload's per-destination runs are 16B-contiguous for the A2A writes
    ATsB = np.stack([np.cos(ang), -np.sin(ang)], axis=2).reshape(H, 64).astype(f16)
    ATqB = np.stack([np.sin(ang), np.cos(ang)], axis=2).reshape(H, 64).astype(f16)
    atsB = np.concatenate([ATsB[0:128], ATsB[128:256]], axis=1)
    atqB = np.concatenate([ATqB[0:128], ATqB[128:256]], axis=1)

    wt = (Wt_w / Wt_w.sum()).reshape(T)
    Wc = (R_w * Ws_w[None, None, None]
          * wt[:, None, None, None, None].astype(f32) * W_SCALE)
    Wr = np.real(Wc).astype(f32)   # [T,U,U,MX,MY] = (t,i,o,kx,ky)
    Wi = np.imag(Wc).astype(f32)

    # w tiles: rows (tb, i); cols (kx*4+kyl)*64 + compW*32 + o ; ky = 4c + kyl
    # [T,U,U,MX,MY] -> per (t,i): [o, kx, ky]
    Wr_t = Wr.transpose(0, 1, 3, 4, 2)   # [t, i, kx, ky, o]
    Wi_t = Wi.transpose(0, 1, 3, 4, 2)
    w_A = np.empty((NC, 128, 8192), f16)
    w_B = np.empty((NC, 64, 8192), f16)
    for c in range(NC):
        kys = slice(4 * c, 4 * c + 4)
        for tb in range(4):
            p, jj = tb // 2, tb % 2
            t = 3 * p + jj
            blk = np.stack([Wr_t[t, :, :, kys, :], Wi_t[t, :, :, kys, :]], axis=3)
            # blk: [i, kx, kyl, compW, o] -> cols (kx, kyl, compW, o)
            w_A[c, tb * 32:(tb + 1) * 32] = blk.reshape(U, 8192).astype(f16)
        for tb in range(2):
            t = 3 * tb + 2
            blk = np.stack([Wr_t[t, :, :, kys, :], Wi_t[t, :, :, kys, :]], axis=3)
            w_B[c, tb * 32:(tb + 1) * 32] = blk.reshape(U, 8192).astype(f16)

    xg = np.arange(MX)[:, None]
    wg = np.arange(W)[None, :]
    ang2 = 2.0 * np.pi * xg * wg / W
    cos2 = np.cos(ang2).astype(f32)
    sin2 = np.sin(ang2).astype(f32)
    cos4 = np.tile(cos2.astype(f16), (4, 1))   # [128, 256]
    sin4 = np.tile(sin2.astype(f16), (4, 1))
    sAB = np.zeros((128, 1024), f16)
    for rep in range(2):
        r0 = rep * 64
        for hc in range(2):
            blkc = cos2[:, hc * 128:(hc + 1) * 128].astype(f16)
            blks = sin2[:, hc * 128:(hc + 1) * 128].astype(f16)
            sAB[r0:r0 + 32, hc * 128:hc * 128 + 128] = blkc
            sAB[r0 + 32:r0 + 64, hc * 128:hc * 128 + 128] = -blks
            sAB[r0:r0 + 32, 512 + hc * 128:512 + hc * 128 + 128] = blks
            sAB[r0 + 32:r0 + 64, 512 + hc * 128:512 + hc * 128 + 128] = blkc
    in_maps = []
    for c in range(NC):
        in_maps.append({
            "xsh": np.ascontiguousarray(xf[c * 3:(c + 1) * 3].reshape(96, H * W)),
            "ats_d": ats, "atsB_d": atsB, "atqB_d": atqB,
            "wA_d": np.ascontiguousarray(w_A[c]),
            "wB_d": np.ascontiguousarray(w_B[c]),
            "cos4_d": cos4, "sin4_d": sin4, "sAB_d": sAB,
        })
    return in_maps


def _host_post(results):
    out = np.empty((B, 1, U, H, W), np.complex64)
    inv = np.float32(1.0 / OUT_DESCALE)
    for c in range(NC):
        arr = np.asarray(results[c]["outp"]).astype(np.float32)  # [16,2,256,256]
        carr = (arr[:, 0] + 1j * arr[:, 1]).astype(np.complex64)
        b, oh = c // 2, c % 2
        for ol in range(CH_PER_CORE):
            out[b, 0, oh * 16 + ol] = carr[ol] * inv
    return out


def kernel(**inputs):
    nc = _get_nc()
    in_maps = _host_prep(inputs["input"], inputs["R_w"], inputs["Ws_w"], inputs["Wt_w"])
    res = bass_utils.run_bass_kernel_spmd(nc, in_maps, core_ids=list(range(NC)))
    return _host_post(res.results)


# revision 37
# speedup vs baseline: 1.9565x; 1.0054x over previous
"""Trainium2 Bass kernel for AutoRegressiveAdaptiveSpectralConv2d (v2).

reference:  f = fft2(x)[..., :32, :32]
            o = einsum('btixy,tioxy->btoxy', f, R_w) * Ws_w
            o = (o * Wt/sum(Wt)).sum(t)            -> [B,1,U,32,32]
            out = ifft2(o, s=(256,256))            -> [B,1,U,256,256] complex64

Device decomposition (8 cores, single SPMD launch, fp16 data / fp32 PSUM):
  phase 1 (truncated DFT, sharded over 24 (b,t) pairs, 3/core = 96 images):
      stage A: PQT[w, (half,kx)] = x^T @ [cos|-sin]  (x chunks stationary)
      stage B: f[(i4,kx), (comp,ky)] = P@[cos|-sin] + (-Q)@[sin|cos]
      4-image groups share one PSUM bank; M=128 stage-B matmuls.
  AllToAll #1 (split in 2: btl {0,1} then btl {2}) -> ky-sharding
  phase 2 (channel mix, 128 xy/core): stationary = f vectors (M=4 b),
      streamed Wr/Wi (stored once - half the HBM of the re/im-packed form),
      4-way PSUM column tiling (tile_position) for array concurrency.
  AllToAll #2 -> (b, o-half) sharding
  phase 3 (zero-padded iFFT, 16 channels/core):
      G-stage: K=32 row-tiled x4; M packs (ch, comp, kx) so all PSUM->SBUF
      copies stay partition-aligned.  H-stage: K=64 packs (comp,kx) so each
      w-column streams once; 2-way row tiling via duplicated tables.
Dummy matmul chains keep the PE HAM clock warm across collective gaps.
Weights pre-scaled by 2^22 on host; host divides output by 2^38.
"""
import sys
import numpy as np

sys.path.insert(0, "/opt/trn_rl_repo")

import concourse.bass as bass
import concourse.bacc as bacc
import concourse.mybir as mybir
import concourse.tile as tile
from concourse import bass_utils

B, T, U, H, W = 4, 6, 32, 256, 256
MX, MY = 32, 32
NC = 8
CH_PER_CORE = 16
W_SCALE = float(2 ** 22)
OUT_DESCALE = float(2 ** 22) * float(H * W)

F16 = mybir.dt.float16
F32 = mybir.dt.float32


def _ap(t, offset, dims):
    """AP on a pool tile with explicit [step, count] dims (tile-relative)."""
    return bass.AP(t[:].tensor, offset, dims)


def build_nc():
    nc = bacc.Bacc("TRN2", target_bir_lowering=False, debug=False, num_devices=NC)

    xsh = nc.dram_tensor("xsh", [96, H * W], F16, kind="ExternalInput")
    ats_d = nc.dram_tensor("ats_d", [128, 128], F16, kind="ExternalInput")
    atsB_d = nc.dram_tensor("atsB_d", [128, 128], F16, kind="ExternalInput")
    atqB_d = nc.dram_tensor("atqB_d", [128, 128], F16, kind="ExternalInput")
    wAll_d = nc.dram_tensor("wAll_d", [192, 8192], F16, kind="ExternalInput")
    cos4_d = nc.dram_tensor("cos4_d", [128, 256], F16, kind="ExternalInput")
    sin4_d = nc.dram_tensor("sin4_d", [128, 256], F16, kind="ExternalInput")
    sAB_d = nc.dram_tensor("sAB_d", [128, 1024], F16, kind="ExternalInput")
    outp = nc.dram_tensor("outp", [CH_PER_CORE, 2, H, W], F16, kind="ExternalOutput")

    with tile.TileContext(nc) as tc:
        with (
            tc.tile_pool(name="dram", bufs=1, space="DRAM") as dram,
            tc.tile_pool(name="consts", bufs=1) as consts,
            tc.tile_pool(name="xpool", bufs=3) as xpool,
            tc.tile_pool(name="pqt", bufs=3) as pqtp,
            tc.tile_pool(name="fsb", bufs=3) as fsbp,
            tc.tile_pool(name="wpool", bufs=1) as wpool,
            tc.tile_pool(name="f2", bufs=1) as f2p,
            tc.tile_pool(name="osb", bufs=1) as osbp,
            tc.tile_pool(name="p3", bufs=1) as p3p,
            tc.tile_pool(name="gsb", bufs=8) as gsbp,
            tc.tile_pool(name="outsb", bufs=3) as outsbp,
        ):
            f_ins = []
            f_outs = []
            for _j in range(3):
                fin = dram.tile([256, 256], F16, tag=f"fin{_j}")
                fout = dram.tile([256, 256], F16, tag=f"fout{_j}")
                f_ins.append(fin)
                f_outs.append(fout)
            o_in = dram.tile([32, 1024], F16)
            o_out = dram.tile([32, 1024], F16)

            # ---- consts + weights + x streaming (sync queue carries x first) ----
            dummy_sb = consts.tile([128, 512], F16)
            nc.vector.memset(dummy_sb[:], 0.25)
            ats = consts.tile([128, 128], F16)
            nc.scalar.dma_start(ats[:], ats_d[:])
            atsB = consts.tile([128, 128], F16)
            nc.scalar.dma_start(atsB[:], atsB_d[:])
            atqB = consts.tile([128, 128], F16)
            nc.scalar.dma_start(atqB[:], atqB_d[:])
            cos4 = consts.tile([128, 256], F16)
            nc.scalar.dma_start(cos4[:], cos4_d[:])
            sin4 = consts.tile([128, 256], F16)
            nc.scalar.dma_start(sin4[:], sin4_d[:])
            sAB = consts.tile([128, 1024], F16)
            nc.scalar.dma_start(sAB[:], sAB_d[:])

            x_tiles = []
            for k in range(6):
                x_sb = xpool.tile([128, 8192], F16, tag="x")
                nc.sync.dma_start(
                    _ap(x_sb, 0, [[8192, 128], [1, 8192]]),
                    bass.AP(xsh, k * 16 * H * W, [[8192, 128], [1, 8192]]),
                )
                x_tiles.append(x_sb)
            w_js = []
            for _j in range(3):
                wj = wpool.tile([64, 8192], F16, tag=f"w{_j}")
                nc.sync.dma_start(
                    wj[:], bass.AP(wAll_d, _j * 64 * 8192,
                                   [[8192, 64], [1, 8192]]))
                w_js.append(wj)


            # ---- PE warm-up (HAM) ----
            pdummy_ctx = tc.tile_pool(name="pdummy", bufs=1, space="PSUM")
            pdummy = pdummy_ctx.__enter__()
            dummy_ps = pdummy.tile([128, 512], F32)
            for _ in range(30):
                nc.tensor.matmul(
                    dummy_ps[:], _ap(dummy_sb, 0, [[512, 128], [1, 128]]),
                    dummy_sb[:], start=True, stop=True)

            # ================= phase 1: truncated DFT =================
            p1a_ctx = tc.tile_pool(name="p1a", bufs=2, space="PSUM")
            p1a = p1a_ctx.__enter__()
            p1b_ctx = tc.tile_pool(name="p1b", bufs=2, space="PSUM")
            p1b = p1b_ctx.__enter__()
            eng = 0
            f_btl = []
            for j in range(3):
                fb = fsbp.tile([128, 512], F16, tag="fsb")
                f_btl.append(fb)
            for k in range(6):
                x_sb = x_tiles[k]
                j = k // 2
                for g4 in range(4):
                    g8 = (k % 2) * 4 + g4          # group index within btl
                    psum_a = p1a.tile([128, 512], F32, tag="a")
                    for i4 in range(4):
                        il = g4 * 4 + i4
                        for wc in range(2):
                            for hc in range(2):
                                # start=True clears the whole bank: only the
                                # first matmul into this tile may set it
                                nc.tensor.matmul(
                                    _ap(psum_a, i4 * 128 + wc * 64,
                                        [[512, 128], [1, 64]]),
                                    _ap(x_sb, il * 512 + hc * 256 + wc * 128,
                                        [[8192, 128], [1, 128]]),
                                    _ap(ats, hc * 64, [[128, 128], [1, 64]]),
                                    start=(i4 == 0 and wc == 0 and hc == 0),
                                    stop=(i4 == 3 and wc == 1 and hc == 1),
                                    skip_group_check=True,
                                )
                    pqt = pqtp.tile([128, 512], F16, tag="pqt")
                    for half in range(2):
                        cp = nc.scalar.copy if (eng % 2 == 0) else nc.vector.tensor_copy
                        cp(
                            _ap(pqt, half * 256,
                                [[512, 128], [128, 2], [32, 4], [1, 32]]),
                            _ap(psum_a, half * 32,
                                [[512, 128], [64, 2], [128, 4], [1, 32]]),
                        )
                        eng += 1
                    psum_b = p1b.tile([128, 64], F32, tag="b")
                    step = 0
                    for half in range(2):
                        rhs_t = atsB if half == 0 else atqB
                        for wc in range(2):
                            nc.tensor.matmul(
                                psum_b[:],
                                _ap(pqt, half * 256 + wc * 128,
                                    [[512, 128], [1, 128]]),
                                _ap(rhs_t, wc * 64, [[128, 128], [1, 64]]),
                                start=(step == 0), stop=(step == 3),
                            )
                            step += 1
                    cp = nc.scalar.copy if (eng % 2 == 0) else nc.vector.tensor_copy
                    cp(_ap(f_btl[j], g8 * 64, [[512, 128], [1, 64]]), psum_b[:])
                    eng += 1
                if k % 2 == 1:
                    # btl j complete: write its A2A payload.  f_sb cols are
                    # (g8, ky, comp) with (kyl, comp) 8-elem runs per dst, so
                    # one 16KB DMA per destination suffices.  Each btl gets
                    # its own AllToAll so the first one absorbs launch skew
                    # while later compute still runs.
                    for dd in range(NC):
                        nc.scalar.dma_start(
                            bass.AP(f_ins[j][:].tensor, dd * 8192,
                                    [[8, 128], [1024, 8], [1, 8]]),
                            _ap(f_btl[j], dd * 8, [[512, 128], [64, 8], [1, 8]]),
                        )
                    nc.gpsimd.collective_compute(
                        "AllToAll", mybir.AluOpType.bypass,
                        replica_groups=[list(range(NC))],
                        ins=[f_ins[j].opt()], outs=[f_outs[j].opt()],
                    )
            p1b_ctx.__exit__(None, None, None)
            p1a_ctx.__exit__(None, None, None)

            # keep PE warm across A2A #1 (anchored on last f copy)
            for d in range(24):
                nc.tensor.matmul(
                    _ap(dummy_ps, 0, [[512, 64], [1, 512]]),
                    _ap(f_btl[2], 448, [[512, 128], [1, 64]]),
                    dummy_sb[:], start=True, stop=True)

            # ================= phase 2: spectral channel mixing =============
            o_sb = osbp.tile([128, 2048], F16)
            nc.vector.memset(o_sb[:], 0)
            fcis = []
            fngs = []
            for _j in range(3):
                fci = f2p.tile([64, 1024], F16, tag=f"fci{_j}")
                fng = f2p.tile([64, 1024], F16, tag=f"fng{_j}")
                fcis.append(fci)
                fngs.append(fng)
            for jj in range(3):
                for p in range(2):
                    nc.scalar.dma_start(
                        _ap(fcis[jj], p * 32 * 1024,
                            [[1024, 32], [256, 4], [1, 256]]),
                        bass.AP(f_outs[jj][:].tensor, p * 8192,
                                [[256, 32], [16384, 4], [1, 256]]),
                    )
                nc.vector.tensor_scalar_mul(fngs[jj][:], fcis[jj][:], -1.0)

            p2_ctx = tc.tile_pool(name="p2ps", bufs=4, space="PSUM")
            p2ps = p2_ctx.__enter__()
            o_ps = []
            for _g in range(4):
                opst = p2ps.tile([128, 512], F32, tag="ops")
                nc.vector.memset(opst[:], 0)
                o_ps.append(opst)
            for rnd in range(3):
                fci, fng, kk = fcis[rnd], fngs[rnd], 64
                w_t = w_js[rnd]
                for g in range(32):
                    for j4 in range(4):
                        xy = g * 4 + j4
                        cb = (xy // 4) * 8 + (xy % 4) * 2
                        lre = _ap(fci, cb, [[1024, kk], [256, 4]])
                        lng = _ap(fng, cb + 1, [[1024, kk], [256, 4]])
                        lim = _ap(fci, cb + 1, [[1024, kk], [256, 4]])
                        rw = _ap(w_t, xy * 64, [[8192, kk], [1, 64]])
                        rwi = _ap(w_t, xy * 64 + 32, [[8192, kk], [1, 32]])
                        rwr = _ap(w_t, xy * 64, [[8192, kk], [1, 32]])
                        ot = o_ps[g // 8]
                        oc = 32 * j4 * 512 + (g % 8) * 64
                        tp = (0, 32 * j4)
                        # start=True clears the WHOLE psum bank, so only the
                        # very first matmul touching each bank may set it.
                        bank_first = (rnd == 0 and g % 8 == 0 and j4 == 0)
                        bank_last = (rnd == 2 and g % 8 == 7 and j4 == 3)
                        nc.tensor.matmul(
                            _ap(ot, oc, [[512, 4], [1, 64]]),
                            lre, rw, start=bank_first, stop=False,
                            tile_position=tp, skip_group_check=True)
                        nc.tensor.matmul(
                            _ap(ot, oc, [[512, 4], [1, 32]]),
                            lng, rwi, start=False, stop=False,
                            tile_position=tp, skip_group_check=True)
                        nc.tensor.matmul(
                            _ap(ot, oc + 32, [[512, 4], [1, 32]]),
                            lim, rwr, start=False, stop=bank_last,
                            tile_position=tp, skip_group_check=True)
            # o_sb cols: g*64 + oh*32 + comp*16 + ol  (oh-major so each dst
            # block reads one contiguous 32-col run per (j4, g)).  The psum
            # banks were memset, so full-width copies are race-clean.
            for gb in range(4):
                for oh in range(2):
                    cp = (nc.scalar.copy if (gb + oh) % 2 == 0
                          else nc.vector.tensor_copy)
                    cp(_ap(o_sb, gb * 512 + oh * 32,
                           [[2048, 128], [64, 8], [16, 2], [1, 16]]),
                       _ap(o_ps[gb], oh * 16,
                           [[512, 128], [64, 8], [32, 2], [1, 16]]))
            p2_ctx.__exit__(None, None, None)
            # o_in block for dst d: [4 rows (j4), 1024 cols (g, comp|ol)].
            # The (b, oh) selection happens here on the send side (SPMD-safe);
            # single-partition reads because partition-strided APs are illegal.
            for d in range(NC):
                b, oh = d // 2, d % 2
                for j4 in range(4):
                    eng_dma = nc.sync if (d + j4) % 2 == 0 else nc.scalar
                    eng_dma.dma_start(
                        bass.AP(o_in[:].tensor, d * 4096 + j4 * 1024,
                                [[1024, 1], [32, 32], [1, 32]]),
                        _ap(o_sb, (32 * j4 + b) * 2048 + oh * 32,
                            [[2048, 1], [64, 32], [1, 32]]),
                    )
            nc.gpsimd.collective_compute(
                "AllToAll", mybir.AluOpType.bypass,
                replica_groups=[list(range(NC))],
                ins=[o_in.opt()], outs=[o_out.opt()],
            )

            # keep PE warm across A2A #2 (anchored on last o copies)
            for d in range(40):
                nc.tensor.matmul(
                    _ap(dummy_ps, 0, [[512, 64], [1, 512]]),
                    _ap(o_sb, 1984, [[2048, 128], [1, 64]]),
                    dummy_sb[:], start=True, stop=True)

            # ================= phase 3: zero-padded iFFT ====================
            # o_out rows: s*4 + j4(=kyl) ; cols g(=kx)*32 + comp*16 + ol
            o3a = p3p.tile([32, 1024], F16)
            for s in range(NC):
                eng_dma = nc.sync if s % 2 == 0 else nc.scalar
                eng_dma.dma_start(
                    _ap(o3a, 4 * s * 1024, [[1024, 4], [32, 32], [16, 2], [1, 16]]),
                    bass.AP(o_out[:].tensor, s * 4096,
                            [[1024, 4], [32, 32], [16, 2], [1, 16]]),
                )
            o3r = p3p.tile([128, 1024], F16)
            for jg in range(4):
                eng_dma = nc.sync if jg % 2 == 0 else nc.scalar
                eng_dma.dma_start(
                    _ap(o3r, jg * 32 * 1024, [[1024, 32], [1, 1024]]),
                    _ap(o3a, 0, [[1024, 32], [1, 1024]]),
                )
            st_p = p3p.tile([128, 1024], F16)
            st_q = p3p.tile([128, 1024], F16)
            for cp_i in range(8):
                jg = cp_i % 4
                base = jg * 32 * 1024
                # st_p: (ch, compG, kx) <- comp_src = compG
                cpe = nc.scalar.copy if (cp_i % 2 == 0) else nc.vector.tensor_copy
                cpe(
                    _ap(st_p, base + cp_i * 128,
                        [[1024, 32], [64, 2], [32, 2], [1, 32]]),
                    _ap(o3r, base + 2 * cp_i,
                        [[1024, 32], [1, 2], [16, 2], [32, 32]]),
                )
                # st_q compG=0 half: -Oi
                nc.scalar.mul(
                    _ap(st_q, base + cp_i * 128,
                        [[1024, 32], [64, 2], [1, 32]]),
                    _ap(o3r, base + 2 * cp_i + 16,
                        [[1024, 32], [1, 2], [32, 32]]),
                    -1.0,
                )
                # st_q compG=1 half: +Or
                nc.vector.tensor_copy(
                    _ap(st_q, base + cp_i * 128 + 32,
                        [[1024, 32], [64, 2], [1, 32]]),
                    _ap(o3r, base + 2 * cp_i,
                        [[1024, 32], [1, 2], [32, 32]]),
                )
            p3g_ctx = tc.tile_pool(name="p3g", bufs=4, space="PSUM")
            p3g = p3g_ctx.__enter__()
            p3h_ctx = tc.tile_pool(name="p3h", bufs=3, space="PSUM")
            p3h = p3h_ctx.__enter__()
            g_tiles = []
            for cp_i in range(8):
                jg = cp_i % 4
                base = jg * 32 * 1024
                gp = p3g.tile([128, 256], F32, tag="gp")
                nc.tensor.matmul(
                    gp[:],
                    _ap(st_p, base + cp_i * 128, [[1024, 32], [1, 128]]),
                    _ap(cos4, jg * 32 * 256, [[256, 32], [1, 256]]),
                    start=True, stop=False, tile_position=(32 * jg, 0))
                nc.tensor.matmul(
                    gp[:],
                    _ap(st_q, base + cp_i * 128, [[1024, 32], [1, 128]]),
                    _ap(sin4, jg * 32 * 256, [[256, 32], [1, 256]]),
                    start=False, stop=True, tile_position=(32 * jg, 0))
                g_sb = gsbp.tile([128, 256], F16, tag="gsb")
                cpe = nc.scalar.copy if (cp_i % 2 == 0) else nc.vector.tensor_copy
                cpe(g_sb[:], gp[:])
                g_tiles.append(g_sb)
            eng = 0
            for cp_i in range(8):
                for par in range(2):
                    ch = cp_i * 2 + par
                    out_sb = outsbp.tile([128, 1024], F16, tag="osb3")
                    for hc in range(2):
                        hp = p3h.tile([128, 512], F32, tag="hp")
                        for variant in range(2):
                            nc.tensor.matmul(
                                _ap(hp, variant * 256, [[512, 128], [1, 256]]),
                                _ap(sAB, par * 64 * 1024 + variant * 512 + hc * 128,
                                    [[1024, 64], [1, 128]]),
                                _ap(g_tiles[cp_i], par * 64 * 256,
                                    [[256, 64], [1, 256]]),
                                start=(variant == 0), stop=(variant == 1),
                                tile_position=(par * 64, 0),
                                skip_group_check=True)
                        # out_sb cols: comp*512 + hc*256 + w (comp-major so the
                        # outp DMA merges to <=3 dims on both sides)
                        cpe = nc.scalar.copy if (eng % 2 == 0) else nc.vector.tensor_copy
                        cpe(_ap(out_sb, hc * 256, [[1024, 128], [512, 2], [1, 256]]),
                            hp[:])
                        eng += 1
                    eng_dma = nc.sync if ch % 2 == 0 else nc.scalar
                    eng_dma.dma_start(
                        bass.AP(outp, ch * 2 * H * W,
                                [[256, 128], [128 * 256, 4], [1, 256]]),
                        _ap(out_sb, 0, [[1024, 128], [256, 4], [1, 256]]),
                    )
            p3h_ctx.__exit__(None, None, None)
            p3g_ctx.__exit__(None, None, None)
            pdummy_ctx.__exit__(None, None, None)
    nc.compile()
    return nc


_NC_CACHE = None


def _get_nc():
    global _NC_CACHE
    if _NC_CACHE is None:
        _NC_CACHE = build_nc()
    return _NC_CACHE


def _host_prep(x, R_w, Ws_w, Wt_w):
    x = np.asarray(x)
    R_w = np.asarray(R_w)
    Ws_w = np.asarray(Ws_w, dtype=np.float32)
    Wt_w = np.asarray(Wt_w, dtype=np.float32)
    f16, f32 = np.float16, np.float32

    xf = x.reshape(B * T, U, H, W).astype(f16)
    # device layout: [chunk 6][partition 128][il 16, hc 2, w 256] per core
    # (fully contiguous DMAs; h = hc*128 + p)

    h = np.arange(H)[:, None]
    k = np.arange(MX)[None, :]
    ang = 2.0 * np.pi * h * k / H
    ATs = np.concatenate([np.cos(ang), -np.sin(ang)], axis=1).astype(f16)
    ats = np.concatenate([ATs[0:128], ATs[128:256]], axis=1)   # [128, 128]
    # stage-B tables have (ky, comp)-interleaved columns so that the f
    # payload's per-destination runs are 16B-contiguous for the A2A writes
    ATsB = np.stack([np.cos(ang), -np.sin(ang)], axis=2).reshape(H, 64).astype(f16)
    ATqB = np.stack([np.sin(ang), np.cos(ang)], axis=2).reshape(H, 64).astype(f16)
    atsB = np.concatenate([ATsB[0:128], ATsB[128:256]], axis=1)
    atqB = np.concatenate([ATqB[0:128], ATqB[128:256]], axis=1)

    wt = (Wt_w / Wt_w.sum()).reshape(T)
    Wc = (R_w * Ws_w[None, None, None]
          * wt[:, None, None, None, None].astype(f32) * W_SCALE)
    Wr = np.real(Wc).astype(f32)   # [T,U,U,MX,MY] = (t,i,o,kx,ky)
    Wi = np.imag(Wc).astype(f32)

    # w tiles: per round j (btl), rows (p, i) with t = 3p + j;
    # cols (kx*4+kyl)*64 + compW*32 + o ; ky = 4c + kyl
    Wr_t = Wr.transpose(0, 1, 3, 4, 2)   # [t, i, kx, ky, o]
    Wi_t = Wi.transpose(0, 1, 3, 4, 2)
    w_all = np.empty((NC, 192, 8192), f16)
    for c in range(NC):
        kys = slice(4 * c, 4 * c + 4)
        for jj in range(3):
            for p in range(2):
                t = 3 * p + jj
                blk = np.stack([Wr_t[t, :, :, kys, :], Wi_t[t, :, :, kys, :]],
                               axis=3)
                # blk: [i, kx, kyl, compW, o] -> cols (kx, kyl, compW, o)
                w_all[c, jj * 64 + p * 32: jj * 64 + (p + 1) * 32] = \
                    blk.reshape(U, 8192).astype(f16)

    xg = np.arange(MX)[:, None]
    wg = np.arange(W)[None, :]
    ang2 = 2.0 * np.pi * xg * wg / W
    cos2 = np.cos(ang2).astype(f32)
    sin2 = np.sin(ang2).astype(f32)
    cos4 = np.tile(cos2.astype(f16), (4, 1))   # [128, 256]
    sin4 = np.tile(sin2.astype(f16), (4, 1))
    sAB = np.zeros((128, 1024), f16)
    for rep in range(2):
        r0 = rep * 64
        for hc in range(2):
            blkc = cos2[:, hc * 128:(hc + 1) * 128].astype(f16)
            blks = sin2[:, hc * 128:(hc + 1) * 128].astype(f16)
            sAB[r0:r0 + 32, hc * 128:hc * 128 + 128] = blkc
            sAB[r0 + 32:r0 + 64, hc * 128:hc * 128 + 128] = -blks
            sAB[r0:r0 + 32, 512 + hc * 128:512 + hc * 128 + 128] = blks
            sAB[r0 + 32:r0 + 64, 512 + hc * 128:512 + hc * 128 + 128] = blkc
    in_maps = []
    for c in range(NC):
        in_maps.append({
            "xsh": np.ascontiguousarray(
                xf[c * 3:(c + 1) * 3].reshape(6, 16, 2, 128, 256)
                .transpose(0, 3, 1, 2, 4).reshape(96, H * W)),
            "ats_d": ats, "atsB_d": atsB, "atqB_d": atqB,
            "wAll_d": np.ascontiguousarray(w_all[c]),
            "cos4_d": cos4, "sin4_d": sin4, "sAB_d": sAB,
        })
    return in_maps


def _host_post(results):
    out = np.empty((B, 1, U, H, W), np.complex64)
    inv = np.float32(1.0 / OUT_DESCALE)
    for c in range(NC):
        arr = np.asarray(results[c]["outp"]).astype(np.float32)  # [16,2,256,256]
        carr = (arr[:, 0] + 1j * arr[:, 1]).astype(np.complex64)
        b, oh = c // 2, c % 2
        for ol in range(CH_PER_CORE):
            out[b, 0, oh * 16 + ol] = carr[ol] * inv
    return out


def kernel(**inputs):
    nc = _get_nc()
    in_maps = _host_prep(inputs["input"], inputs["R_w"], inputs["Ws_w"], inputs["Wt_w"])
    res = bass_utils.run_bass_kernel_spmd(nc, in_maps, core_ids=list(range(NC)))
    return _host_post(res.results)


# revision 38
# speedup vs baseline: 1.9692x; 1.0065x over previous
"""Trainium2 Bass kernel for AutoRegressiveAdaptiveSpectralConv2d (v2).

reference:  f = fft2(x)[..., :32, :32]
            o = einsum('btixy,tioxy->btoxy', f, R_w) * Ws_w
            o = (o * Wt/sum(Wt)).sum(t)            -> [B,1,U,32,32]
            out = ifft2(o, s=(256,256))            -> [B,1,U,256,256] complex64

Device decomposition (8 cores, single SPMD launch, fp16 data / fp32 PSUM):
  phase 1 (truncated DFT, sharded over 24 (b,t) pairs, 3/core = 96 images):
      stage A: PQT[w, (half,kx)] = x^T @ [cos|-sin]  (x chunks stationary)
      stage B: f[(i4,kx), (comp,ky)] = P@[cos|-sin] + (-Q)@[sin|cos]
      4-image groups share one PSUM bank; M=128 stage-B matmuls.
  AllToAll #1 (split in 2: btl {0,1} then btl {2}) -> ky-sharding
  phase 2 (channel mix, 128 xy/core): stationary = f vectors (M=4 b),
      streamed Wr/Wi (stored once - half the HBM of the re/im-packed form),
      4-way PSUM column tiling (tile_position) for array concurrency.
  AllToAll #2 -> (b, o-half) sharding
  phase 3 (zero-padded iFFT, 16 channels/core):
      G-stage: K=32 row-tiled x4; M packs (ch, comp, kx) so all PSUM->SBUF
      copies stay partition-aligned.  H-stage: K=64 packs (comp,kx) so each
      w-column streams once; 2-way row tiling via duplicated tables.
Dummy matmul chains keep the PE HAM clock warm across collective gaps.
Weights pre-scaled by 2^22 on host; host divides output by 2^38.
"""
import sys
import numpy as np

sys.path.insert(0, "/opt/trn_rl_repo")

import concourse.bass as bass
import concourse.bacc as bacc
import concourse.mybir as mybir
import concourse.tile as tile
from concourse import bass_utils

B, T, U, H, W = 4, 6, 32, 256, 256
MX, MY = 32, 32
NC = 8
CH_PER_CORE = 16
W_SCALE = float(2 ** 22)
OUT_DESCALE = float(2 ** 22) * float(H * W)

F16 = mybir.dt.float16
F32 = mybir.dt.float32


def _ap(t, offset, dims):
    """AP on a pool tile with explicit [step, count] dims (tile-relative)."""
    return bass.AP(t[:].tensor, offset, dims)


def build_nc():
    nc = bacc.Bacc("TRN2", target_bir_lowering=False, debug=False, num_devices=NC)

    xsh = nc.dram_tensor("xsh", [96, H * W], F16, kind="ExternalInput")
    ats_d = nc.dram_tensor("ats_d", [128, 128], F16, kind="ExternalInput")
    atsB_d = nc.dram_tensor("atsB_d", [128, 128], F16, kind="ExternalInput")
    atqB_d = nc.dram_tensor("atqB_d", [128, 128], F16, kind="ExternalInput")
    wAll_d = nc.dram_tensor("wAll_d", [192, 8192], F16, kind="ExternalInput")
    cos4_d = nc.dram_tensor("cos4_d", [128, 256], F16, kind="ExternalInput")
    sin4_d = nc.dram_tensor("sin4_d", [128, 256], F16, kind="ExternalInput")
    sAB_d = nc.dram_tensor("sAB_d", [128, 1024], F16, kind="ExternalInput")
    outp = nc.dram_tensor("outp", [CH_PER_CORE, 2, H, W], F16, kind="ExternalOutput")

    with tile.TileContext(nc) as tc:
        with (
            tc.tile_pool(name="dram", bufs=1, space="DRAM") as dram,
            tc.tile_pool(name="consts", bufs=1) as consts,
            tc.tile_pool(name="xpool", bufs=3) as xpool,
            tc.tile_pool(name="pqt", bufs=3) as pqtp,
            tc.tile_pool(name="fsb", bufs=3) as fsbp,
            tc.tile_pool(name="wpool", bufs=1) as wpool,
            tc.tile_pool(name="f2", bufs=1) as f2p,
            tc.tile_pool(name="osb", bufs=1) as osbp,
            tc.tile_pool(name="p3", bufs=1) as p3p,
            tc.tile_pool(name="gsb", bufs=8) as gsbp,
            tc.tile_pool(name="outsb", bufs=3) as outsbp,
        ):
            f_ins = []
            f_outs = []
            for _j in range(3):
                fin = dram.tile([256, 256], F16, tag=f"fin{_j}")
                fout = dram.tile([256, 256], F16, tag=f"fout{_j}")
                f_ins.append(fin)
                f_outs.append(fout)
            o_in = dram.tile([32, 1024], F16)
            o_out = dram.tile([32, 1024], F16)

            # ---- consts + weights + x streaming (sync queue carries x first) ----
            dummy_sb = consts.tile([128, 512], F16)
            nc.vector.memset(dummy_sb[:], 0.25)
            ats = consts.tile([128, 128], F16)
            nc.scalar.dma_start(ats[:], ats_d[:])
            atsB = consts.tile([128, 128], F16)
            nc.scalar.dma_start(atsB[:], atsB_d[:])
            atqB = consts.tile([128, 128], F16)
            nc.scalar.dma_start(atqB[:], atqB_d[:])
            cos4 = consts.tile([128, 256], F16)
            nc.scalar.dma_start(cos4[:], cos4_d[:])
            sin4 = consts.tile([128, 256], F16)
            nc.scalar.dma_start(sin4[:], sin4_d[:])
            sAB = consts.tile([128, 1024], F16)
            nc.scalar.dma_start(sAB[:], sAB_d[:])

            x_tiles = []
            for k in range(6):
                x_sb = xpool.tile([128, 8192], F16, tag="x")
                nc.sync.dma_start(
                    _ap(x_sb, 0, [[8192, 128], [1, 8192]]),
                    bass.AP(xsh, k * 16 * H * W, [[8192, 128], [1, 8192]]),
                )
                x_tiles.append(x_sb)
            w_js = []
            for _j in range(3):
                wj = wpool.tile([64, 8192], F16, tag=f"w{_j}")
                nc.sync.dma_start(
                    wj[:], bass.AP(wAll_d, _j * 64 * 8192,
                                   [[8192, 64], [1, 8192]]))
                w_js.append(wj)


            # ---- PE warm-up (HAM) ----
            pdummy_ctx = tc.tile_pool(name="pdummy", bufs=1, space="PSUM")
            pdummy = pdummy_ctx.__enter__()
            dummy_ps = pdummy.tile([128, 512], F32)
            for _ in range(30):
                nc.tensor.matmul(
                    dummy_ps[:], _ap(dummy_sb, 0, [[512, 128], [1, 128]]),
                    dummy_sb[:], start=True, stop=True)

            # ================= phase 1: truncated DFT =================
            p1a_ctx = tc.tile_pool(name="p1a", bufs=2, space="PSUM")
            p1a = p1a_ctx.__enter__()
            p1b_ctx = tc.tile_pool(name="p1b", bufs=2, space="PSUM")
            p1b = p1b_ctx.__enter__()
            eng = 0
            f_btl = []
            for j in range(3):
                fb = fsbp.tile([128, 512], F16, tag="fsb")
                f_btl.append(fb)
            for k in range(6):
                x_sb = x_tiles[k]
                j = k // 2
                for g4 in range(4):
                    g8 = (k % 2) * 4 + g4          # group index within btl
                    psum_a = p1a.tile([128, 512], F32, tag="a")
                    for i4 in range(4):
                        il = g4 * 4 + i4
                        for wc in range(2):
                            for hc in range(2):
                                # start=True clears the whole bank: only the
                                # first matmul into this tile may set it
                                nc.tensor.matmul(
                                    _ap(psum_a, i4 * 128 + wc * 64,
                                        [[512, 128], [1, 64]]),
                                    _ap(x_sb, il * 512 + hc * 256 + wc * 128,
                                        [[8192, 128], [1, 128]]),
                                    _ap(ats, hc * 64, [[128, 128], [1, 64]]),
                                    start=(i4 == 0 and wc == 0 and hc == 0),
                                    stop=(i4 == 3 and wc == 1 and hc == 1),
                                    skip_group_check=True,
                                )
                    pqt = pqtp.tile([128, 512], F16, tag="pqt")
                    for half in range(2):
                        cp = nc.vector.tensor_copy if (eng % 2 == 0) else nc.scalar.copy
                        cp(
                            _ap(pqt, half * 256,
                                [[512, 128], [128, 2], [32, 4], [1, 32]]),
                            _ap(psum_a, half * 32,
                                [[512, 128], [64, 2], [128, 4], [1, 32]]),
                        )
                        eng += 1
                    psum_b = p1b.tile([128, 64], F32, tag="b")
                    step = 0
                    for half in range(2):
                        rhs_t = atsB if half == 0 else atqB
                        for wc in range(2):
                            nc.tensor.matmul(
                                psum_b[:],
                                _ap(pqt, half * 256 + wc * 128,
                                    [[512, 128], [1, 128]]),
                                _ap(rhs_t, wc * 64, [[128, 128], [1, 64]]),
                                start=(step == 0), stop=(step == 3),
                            )
                            step += 1
                    cp = nc.scalar.copy if (eng % 2 == 0) else nc.vector.tensor_copy
                    cp(_ap(f_btl[j], g8 * 64, [[512, 128], [1, 64]]), psum_b[:])
                    eng += 1
                if k % 2 == 1:
                    # btl j complete: write its A2A payload.  f_sb cols are
                    # (g8, ky, comp) with (kyl, comp) 8-elem runs per dst, so
                    # one 16KB DMA per destination suffices.  Each btl gets
                    # its own AllToAll so the first one absorbs launch skew
                    # while later compute still runs.
                    for dd in range(NC):
                        eng_dma = nc.sync if dd % 2 == 0 else nc.scalar
                        eng_dma.dma_start(
                            bass.AP(f_ins[j][:].tensor, dd * 8192,
                                    [[8, 128], [1024, 8], [1, 8]]),
                            _ap(f_btl[j], dd * 8, [[512, 128], [64, 8], [1, 8]]),
                        )
                    nc.gpsimd.collective_compute(
                        "AllToAll", mybir.AluOpType.bypass,
                        replica_groups=[list(range(NC))],
                        ins=[f_ins[j].opt()], outs=[f_outs[j].opt()],
                    )
            p1b_ctx.__exit__(None, None, None)
            p1a_ctx.__exit__(None, None, None)

            # keep PE warm across A2A #1 (anchored on last f copy)
            for d in range(24):
                nc.tensor.matmul(
                    _ap(dummy_ps, 0, [[512, 64], [1, 512]]),
                    _ap(f_btl[2], 448, [[512, 128], [1, 64]]),
                    dummy_sb[:], start=True, stop=True)

            # ================= phase 2: spectral channel mixing =============
            o_sb = osbp.tile([128, 2048], F16)
            nc.vector.memset(o_sb[:], 0)
            fcis = []
            fngs = []
            for _j in range(3):
                fci = f2p.tile([64, 1024], F16, tag=f"fci{_j}")
                fng = f2p.tile([64, 1024], F16, tag=f"fng{_j}")
                fcis.append(fci)
                fngs.append(fng)
            for jj in range(3):
                for p in range(2):
                    nc.scalar.dma_start(
                        _ap(fcis[jj], p * 32 * 1024,
                            [[1024, 32], [256, 4], [1, 256]]),
                        bass.AP(f_outs[jj][:].tensor, p * 8192,
                                [[256, 32], [16384, 4], [1, 256]]),
                    )
                nc.vector.tensor_scalar_mul(fngs[jj][:], fcis[jj][:], -1.0)

            p2_ctx = tc.tile_pool(name="p2ps", bufs=4, space="PSUM")
            p2ps = p2_ctx.__enter__()
            o_ps = []
            for _g in range(4):
                opst = p2ps.tile([128, 512], F32, tag="ops")
                nc.vector.memset(opst[:], 0)
                o_ps.append(opst)
            for rnd in range(3):
                fci, fng, kk = fcis[rnd], fngs[rnd], 64
                w_t = w_js[rnd]
                for g in range(32):
                    for j4 in range(4):
                        xy = g * 4 + j4
                        cb = (xy // 4) * 8 + (xy % 4) * 2
                        lre = _ap(fci, cb, [[1024, kk], [256, 4]])
                        lng = _ap(fng, cb + 1, [[1024, kk], [256, 4]])
                        lim = _ap(fci, cb + 1, [[1024, kk], [256, 4]])
                        rw = _ap(w_t, xy * 64, [[8192, kk], [1, 64]])
                        rwi = _ap(w_t, xy * 64 + 32, [[8192, kk], [1, 32]])
                        rwr = _ap(w_t, xy * 64, [[8192, kk], [1, 32]])
                        ot = o_ps[g // 8]
                        oc = 32 * j4 * 512 + (g % 8) * 64
                        tp = (0, 32 * j4)
                        # start=True clears the WHOLE psum bank, so only the
                        # very first matmul touching each bank may set it.
                        bank_first = (rnd == 0 and g % 8 == 0 and j4 == 0)
                        bank_last = (rnd == 2 and g % 8 == 7 and j4 == 3)
                        nc.tensor.matmul(
                            _ap(ot, oc, [[512, 4], [1, 64]]),
                            lre, rw, start=bank_first, stop=False,
                            tile_position=tp, skip_group_check=True)
                        nc.tensor.matmul(
                            _ap(ot, oc, [[512, 4], [1, 32]]),
                            lng, rwi, start=False, stop=False,
                            tile_position=tp, skip_group_check=True)
                        nc.tensor.matmul(
                            _ap(ot, oc + 32, [[512, 4], [1, 32]]),
                            lim, rwr, start=False, stop=bank_last,
                            tile_position=tp, skip_group_check=True)
            # o_sb cols: g*64 + oh*32 + comp*16 + ol  (oh-major so each dst
            # block reads one contiguous 32-col run per (j4, g)).  The psum
            # banks were memset, so full-width copies are race-clean.
            for gb in range(4):
                for oh in range(2):
                    cp = (nc.scalar.copy if (gb + oh) % 2 == 0
                          else nc.vector.tensor_copy)
                    cp(_ap(o_sb, gb * 512 + oh * 32,
                           [[2048, 128], [64, 8], [16, 2], [1, 16]]),
                       _ap(o_ps[gb], oh * 16,
                           [[512, 128], [64, 8], [32, 2], [1, 16]]))
            p2_ctx.__exit__(None, None, None)
            # o_in block for dst d: [4 rows (j4), 1024 cols (g, comp|ol)].
            # The (b, oh) selection happens here on the send side (SPMD-safe);
            # single-partition reads because partition-strided APs are illegal.
            for d in range(NC):
                b, oh = d // 2, d % 2
                for j4 in range(4):
                    eng_dma = nc.sync if (d + j4) % 2 == 0 else nc.scalar
                    eng_dma.dma_start(
                        bass.AP(o_in[:].tensor, d * 4096 + j4 * 1024,
                                [[1024, 1], [32, 32], [1, 32]]),
                        _ap(o_sb, (32 * j4 + b) * 2048 + oh * 32,
                            [[2048, 1], [64, 32], [1, 32]]),
                    )
            nc.gpsimd.collective_compute(
                "AllToAll", mybir.AluOpType.bypass,
                replica_groups=[list(range(NC))],
                ins=[o_in.opt()], outs=[o_out.opt()],
            )

            # keep PE warm across A2A #2 (anchored on last o copies)
            for d in range(40):
                nc.tensor.matmul(
                    _ap(dummy_ps, 0, [[512, 64], [1, 512]]),
                    _ap(o_sb, 1984, [[2048, 128], [1, 64]]),
                    dummy_sb[:], start=True, stop=True)

            # ================= phase 3: zero-padded iFFT ====================
            # o_out rows: s*4 + j4(=kyl) ; cols g(=kx)*32 + comp*16 + ol
            o3a = p3p.tile([32, 1024], F16)
            for s in range(NC):
                eng_dma = nc.sync if s % 2 == 0 else nc.scalar
                eng_dma.dma_start(
                    _ap(o3a, 4 * s * 1024, [[1024, 4], [32, 32], [16, 2], [1, 16]]),
                    bass.AP(o_out[:].tensor, s * 4096,
                            [[1024, 4], [32, 32], [16, 2], [1, 16]]),
                )
            o3r = p3p.tile([128, 1024], F16)
            for jg in range(4):
                eng_dma = nc.sync if jg % 2 == 0 else nc.scalar
                eng_dma.dma_start(
                    _ap(o3r, jg * 32 * 1024, [[1024, 32], [1, 1024]]),
                    _ap(o3a, 0, [[1024, 32], [1, 1024]]),
                )
            st_p = p3p.tile([128, 1024], F16)
            st_q = p3p.tile([128, 1024], F16)
            for cp_i in range(8):
                jg = cp_i % 4
                base = jg * 32 * 1024
                # st_p: (ch, compG, kx) <- comp_src = compG
                cpe = nc.scalar.copy if (cp_i % 2 == 0) else nc.vector.tensor_copy
                cpe(
                    _ap(st_p, base + cp_i * 128,
                        [[1024, 32], [64, 2], [32, 2], [1, 32]]),
                    _ap(o3r, base + 2 * cp_i,
                        [[1024, 32], [1, 2], [16, 2], [32, 32]]),
                )
                # st_q compG=0 half: -Oi
                nc.scalar.mul(
                    _ap(st_q, base + cp_i * 128,
                        [[1024, 32], [64, 2], [1, 32]]),
                    _ap(o3r, base + 2 * cp_i + 16,
                        [[1024, 32], [1, 2], [32, 32]]),
                    -1.0,
                )
                # st_q compG=1 half: +Or
                nc.vector.tensor_copy(
                    _ap(st_q, base + cp_i * 128 + 32,
                        [[1024, 32], [64, 2], [1, 32]]),
                    _ap(o3r, base + 2 * cp_i,
                        [[1024, 32], [1, 2], [32, 32]]),
                )
            p3g_ctx = tc.tile_pool(name="p3g", bufs=4, space="PSUM")
            p3g = p3g_ctx.__enter__()
            p3h_ctx = tc.tile_pool(name="p3h", bufs=3, space="PSUM")
            p3h = p3h_ctx.__enter__()
            g_tiles = []
            for cp_i in range(8):
                jg = cp_i % 4
                base = jg * 32 * 1024
                gp = p3g.tile([128, 256], F32, tag="gp")
                nc.tensor.matmul(
                    gp[:],
                    _ap(st_p, base + cp_i * 128, [[1024, 32], [1, 128]]),
                    _ap(cos4, jg * 32 * 256, [[256, 32], [1, 256]]),
                    start=True, stop=False, tile_position=(32 * jg, 0))
                nc.tensor.matmul(
                    gp[:],
                    _ap(st_q, base + cp_i * 128, [[1024, 32], [1, 128]]),
                    _ap(sin4, jg * 32 * 256, [[256, 32], [1, 256]]),
                    start=False, stop=True, tile_position=(32 * jg, 0))
                g_sb = gsbp.tile([128, 256], F16, tag="gsb")
                cpe = nc.scalar.copy if (cp_i % 2 == 0) else nc.vector.tensor_copy
                cpe(g_sb[:], gp[:])
                g_tiles.append(g_sb)
            eng = 0
            for cp_i in range(8):
                for par in range(2):
                    ch = cp_i * 2 + par
                    out_sb = outsbp.tile([128, 1024], F16, tag="osb3")
                    for hc in range(2):
                        hp = p3h.tile([128, 512], F32, tag="hp")
                        for variant in range(2):
                            nc.tensor.matmul(
                                _ap(hp, variant * 256, [[512, 128], [1, 256]]),
                                _ap(sAB, par * 64 * 1024 + variant * 512 + hc * 128,
                                    [[1024, 64], [1, 128]]),
                                _ap(g_tiles[cp_i], par * 64 * 256,
                                    [[256, 64], [1, 256]]),
                                start=(variant == 0), stop=(variant == 1),
                                tile_position=(par * 64, 0),
                                skip_group_check=True)
                        # out_sb cols: comp*512 + hc*256 + w (comp-major so the
                        # outp DMA merges to <=3 dims on both sides)
                        cpe = nc.scalar.copy if (eng % 2 == 0) else nc.vector.tensor_copy
                        cpe(_ap(out_sb, hc * 256, [[1024, 128], [512, 2], [1, 256]]),
                            hp[:])
                        eng += 1
                    eng_dma = nc.sync if ch % 2 == 0 else nc.scalar
                    eng_dma.dma_start(
                        bass.AP(outp, ch * 2 * H * W,
                                [[256, 128], [128 * 256, 4], [1, 256]]),
                        _ap(out_sb, 0, [[1024, 128], [256, 4], [1, 256]]),
                    )
            p3h_ctx.__exit__(None, None, None)
            p3g_ctx.__exit__(None, None, None)
            pdummy_ctx.__exit__(None, None, None)
    nc.compile()
    return nc


_NC_CACHE = None


def _get_nc():
    global _NC_CACHE
    if _NC_CACHE is None:
        _NC_CACHE = build_nc()
    return _NC_CACHE


def _host_prep(x, R_w, Ws_w, Wt_w):
    x = np.asarray(x)
    R_w = np.asarray(R_w)
    Ws_w = np.asarray(Ws_w, dtype=np.float32)
    Wt_w = np.asarray(Wt_w, dtype=np.float32)
    f16, f32 = np.float16, np.float32

    xf = x.reshape(B * T, U, H, W).astype(f16)
    # device layout: [chunk 6][partition 128][il 16, hc 2, w 256] per core
    # (fully contiguous DMAs; h = hc*128 + p)

    h = np.arange(H)[:, None]
    k = np.arange(MX)[None, :]
    ang = 2.0 * np.pi * h * k / H
    ATs = np.concatenate([np.cos(ang), -np.sin(ang)], axis=1).astype(f16)
    ats = np.concatenate([ATs[0:128], ATs[128:256]], axis=1)   # [128, 128]
    # stage-B tables have (ky, comp)-interleaved columns so that the f
    # payload's per-destination runs are 16B-contiguous for the A2A writes
    ATsB = np.stack([np.cos(ang), -np.sin(ang)], axis=2).reshape(H, 64).astype(f16)
    ATqB = np.stack([np.sin(ang), np.cos(ang)], axis=2).reshape(H, 64).astype(f16)
    atsB = np.concatenate([ATsB[0:128], ATsB[128:256]], axis=1)
    atqB = np.concatenate([ATqB[0:128], ATqB[128:256]], axis=1)

    wt = (Wt_w / Wt_w.sum()).reshape(T)
    Wc = (R_w * Ws_w[None, None, None]
          * wt[:, None, None, None, None].astype(f32) * W_SCALE)
    Wr = np.real(Wc).astype(f32)   # [T,U,U,MX,MY] = (t,i,o,kx,ky)
    Wi = np.imag(Wc).astype(f32)

    # w tiles: per round j (btl), rows (p, i) with t = 3p + j;
    # cols (kx*4+kyl)*64 + compW*32 + o ; ky = 4c + kyl
    Wr_t = Wr.transpose(0, 1, 3, 4, 2)   # [t, i, kx, ky, o]
    Wi_t = Wi.transpose(0, 1, 3, 4, 2)
    w_all = np.empty((NC, 192, 8192), f16)
    for c in range(NC):
        kys = slice(4 * c, 4 * c + 4)
        for jj in range(3):
            for p in range(2):
                t = 3 * p + jj
                blk = np.stack([Wr_t[t, :, :, kys, :], Wi_t[t, :, :, kys, :]],
                               axis=3)
                # blk: [i, kx, kyl, compW, o] -> cols (kx, kyl, compW, o)
                w_all[c, jj * 64 + p * 32: jj * 64 + (p + 1) * 32] = \
                    blk.reshape(U, 8192).astype(f16)

    xg = np.arange(MX)[:, None]
    wg = np.arange(W)[None, :]
    ang2 = 2.0 * np.pi * xg * wg / W
    cos2 = np.cos(ang2).astype(f32)
    sin2 = np.sin(ang2).astype(f32)
    cos4 = np.tile(cos2.astype(f16), (4, 1))   # [128, 256]
    sin4 = np.tile(sin2.astype(f16), (4, 1))
    sAB = np.zeros((128, 1024), f16)
    for rep in range(2):
        r0 = rep * 64
        for hc in range(2):
            blkc = cos2[:, hc * 128:(hc + 1) * 128].astype(f16)
            blks = sin2[:, hc * 128:(hc + 1) * 128].astype(f16)
            sAB[r0:r0 + 32, hc * 128:hc * 128 + 128] = blkc
            sAB[r0 + 32:r0 + 64, hc * 128:hc * 128 + 128] = -blks
            sAB[r0:r0 + 32, 512 + hc * 128:512 + hc * 128 + 128] = blks
            sAB[r0 + 32:r0 + 64, 512 + hc * 128:512 + hc * 128 + 128] = blkc
    in_maps = []
    for c in range(NC):
        in_maps.append({
            "xsh": np.ascontiguousarray(
                xf[c * 3:(c + 1) * 3].reshape(6, 16, 2, 128, 256)
                .transpose(0, 3, 1, 2, 4).reshape(96, H * W)),
            "ats_d": ats, "atsB_d": atsB, "atqB_d": atqB,
            "wAll_d": np.ascontiguousarray(w_all[c]),
            "cos4_d": cos4, "sin4_d": sin4, "sAB_d": sAB,
        })
    return in_maps


def _host_post(results):
    out = np.empty((B, 1, U, H, W), np.complex64)
    inv = np.float32(1.0 / OUT_DESCALE)
    for c in range(NC):
        arr = np.asarray(results[c]["outp"]).astype(np.float32)  # [16,2,256,256]
        carr = (arr[:, 0] + 1j * arr[:, 1]).astype(np.complex64)
        b, oh = c // 2, c % 2
        for ol in range(CH_PER_CORE):
            out[b, 0, oh * 16 + ol] = carr[ol] * inv
    return out


def kernel(**inputs):
    nc = _get_nc()
    in_maps = _host_prep(inputs["input"], inputs["R_w"], inputs["Ws_w"], inputs["Wt_w"])
    res = bass_utils.run_bass_kernel_spmd(nc, in_maps, core_ids=list(range(NC)))
    return _host_post(res.results)
